# revision 15
# baseline (speedup 1.0000x reference)
"""Trainium2 Bass kernel for nn_BaseRGCNHetero (3-layer heterogeneous RGCN).

Strategy (8 NeuronCores, SPMD):
  - Destination-shard the nodes: core c owns rows [c*N/8, (c+1)*N/8) of every
    node type; all edges whose dst is in the shard are processed there, so
    per-relation aggregates need no cross-core reduction.
  - Aggregate-first algebra: agg[dst] = (sum_{e->dst} h[src]) @ W_r * inv_deg,
    sharing one bf16 gather table per source ntype (drug, gene) per layer.
  - After each layer the drug/gene h-shards are AllGathered (bf16) into
    per-core DRAM gather tables for the next layer.  Relations are ordered
    ddr -> dg -> gg -> dd -> gd so each ntype's activation + AllGather fires
    as early as possible and overlaps the remaining relations' gathers.
  - Segment sums: host lays edges out as a padded, degree-bucketed slot
    stream per (relation, index bank, 128-dst window, 8-dst subgroup).
    dma_gather (bf16, transpose=True) produces feature-major tiles; VectorE
    tensor_reduce over the innermost (slot) axis yields segment sums.  Pad
    slots point at an all-zero table row.  Tables over 32768 rows split into
    two int16 index banks; each bank gets its own per-window degree sort so
    subgroup depth padding stays low.
  - Per (relation, bank, window): a one-hot "unpermute * inv_deg" matrix is
    built by a fused tensor_scalar(is_equal, mult); two matmuls apply W_r and
    the window permutation back to natural dst order, accumulating into a
    feature-major fp32 SBUF accumulator (bank contributions sum there); the
    self-loop h @ L is one more matmul; bias+relu is a fused ScalarE
    activation per window.
"""
import sys
import types
import numpy as np
import ml_dtypes
from contextlib import ExitStack

import concourse.bass as bass
import concourse.bacc as bacc
import concourse.tile as tile
from concourse import mybir, library_config

BF16 = ml_dtypes.bfloat16
P = 128
SUBG = 16          # dsts per reduce subgroup
NSUB = P // SUBG   # subgroups per window
GCAP = 8192        # target max slots per dma_gather

CFG = dict(
    N={"drug": 20000, "gene": 50000, "disease": 10000},
    MOD={"drug": 1024, "gene": 768, "disease": 512},
    D_IN=128, D_H=128, D_OUT=64,
    RELS=[("drug", "disease", "dd"), ("drug", "drug", "ddr"),
          ("drug", "gene", "dg"), ("gene", "disease", "gd"),
          ("gene", "gene", "gg")],
    NCORE=8,
)

NTYPES = ("drug", "gene", "disease")
SRC_NTYPES = ("drug", "gene")
# processing order inside a layer: drug-dst first (unlocks drug AllGather),
# then gene-dst, then disease-dst (no AllGather needed).
REL_ORDER = ["ddr", "dg", "gg", "dd", "gd"]


# ---------------------------------------------------------------------------
# host-side preprocessing
# ---------------------------------------------------------------------------

def _pack_idx(stream):
    """int array (len % 128 == 0) -> dma_gather idx layout [128, len/16] int16:
    idx i at (i%16, i//16), replicated across the 8 groups of 16 partitions."""
    n = stream.size
    v = stream.astype(np.int16).reshape(n // 16, 16).T
    return np.tile(v, (8, 1))


def preprocess(cfg, inputs):
    ncore = cfg["NCORE"]
    shard = {nt: cfg["N"][nt] // ncore for nt in NTYPES}
    nw = {nt: -(-shard[nt] // P) for nt in NTYPES}

    S = dict(cfg=cfg, nw=nw, shard=shard, rels=[])
    percore = [dict() for _ in range(ncore)]

    BANK = 32768
    for r, (snt, dnt, tag) in enumerate(cfg["RELS"]):
        src = np.asarray(inputs["e_" + tag + "_s"]).astype(np.int64)
        dst = np.asarray(inputs["e_" + tag + "_d"]).astype(np.int64)
        trows = cfg["N"][snt] + 2
        # table row 0 is all-zero (pad target); banks split the int16 idx range
        if trows <= BANK:
            banks = [(0, trows)]
        else:
            banks = [(0, BANK), (BANK - 1, trows)]  # overlap row BANK-1 unused;
            # bank1 pad slots use relative row 0 -> absolute BANK-1 (a real
            # node!), so bank1 pads instead point at relative (trows-1-b0)
            # which is the trailing all-zero row.
        nbank = len(banks)
        NW = nw[dnt]
        dsh = shard[dnt]
        npad = NW * P

        core_of = dst // dsh
        deg_all = np.bincount(dst, minlength=cfg["N"][dnt]).astype(np.int64)
        row_all = src + 1
        bank_of = np.zeros(src.size, np.int64)
        for b, (b0, b1) in enumerate(banks):
            bank_of[(row_all >= b0 + (1 if b else 0)) & (row_all < b1)] = b
        pad_rel = [(trows - 1) - b0 for b0, b1 in banks]  # trailing zero row
        pad_rel[0] = 0                                    # leading zero row

        # per-bank, per-core window-local orderings by per-bank degree
        orders = np.zeros((nbank, ncore, npad), np.int64)
        perm_cols = np.zeros((nbank, ncore, NW, P), np.int32)
        invdeg_cols = np.zeros((nbank, ncore, NW, P), np.float32)
        dcnt = np.zeros((nbank, ncore, NW, P), np.int64)
        dn = np.arange(npad)
        for c in range(ncore):
            m = core_of == c
            ld_all = dst[m] - c * dsh
            deg_pad = np.zeros(npad, np.int64)
            deg_pad[:dsh] = deg_all[c * dsh:(c + 1) * dsh]
            ivfull = (1.0 / np.maximum(deg_pad, 1.0)).astype(np.float32)
            ivfull[dsh:] = 0.0
            for b in range(nbank):
                cb = np.bincount(ld_all[bank_of[m] == b], minlength=npad)
                order = np.lexsort((dn, -cb, dn // P))
                orders[b, c] = order
                perm_cols[b, c] = (order % P).reshape(NW, P)
                iv = ivfull[order].copy()
                iv[cb[order] == 0] = 0.0   # no bank-b edges -> contribute 0
                invdeg_cols[b, c] = iv.reshape(NW, P)
                dcnt[b, c] = cb[order].reshape(NW, P)

        # subgroup depths, common across cores; window block sizes % 128
        dq = np.zeros((NW, NSUB, nbank), np.int64)
        for q in range(NSUB):
            dq[:, q, :] = dcnt[:, :, :, q * SUBG:(q + 1) * SUBG].max(
                axis=(1, 3)).T
        for b in range(nbank):
            dq[:, NSUB - 1, b] += (-dq[:, :, b].sum(axis=1)) % (P // SUBG)

        # block layout: bank-major, then window, subgroup; greedy gathers
        blocks = []
        OFF = np.full((NW, NSUB, nbank), -1, np.int64)
        off = 0
        gathers = []
        for b in range(nbank):
            gstart, gslots = off, 0
            for w in range(NW):
                wslots = int(dq[w, :, b].sum()) * SUBG
                if wslots == 0:
                    continue
                if gslots + wslots > GCAP and gslots > 0:
                    gathers.append((b, gstart, gslots))
                    gstart, gslots = off, 0
                for q in range(NSUB):
                    if dq[w, q, b] > 0:
                        blocks.append((w, b, q, int(dq[w, q, b]), off))
                        OFF[w, q, b] = off
                        off += int(dq[w, q, b]) * SUBG
                gslots += wslots
            if gslots > 0:
                gathers.append((b, gstart, gslots))
        nslots = max(off, P)
        maxg = max((g[2] for g in gathers), default=P)

        for c in range(ncore):
            stream = np.zeros(nslots, np.int64)
            for (w, b, q, d, o) in blocks:
                stream[o:o + d * SUBG] = pad_rel[b]
            m = core_of == c
            sm_row = row_all[m]
            sm_bank = bank_of[m]
            ld = dst[m] - c * dsh
            for b, (b0, b1) in enumerate(banks):
                inb = sm_bank == b
                if not inb.any():
                    continue
                rel_row = sm_row[inb] - b0
                dp_of = np.zeros(npad, np.int64)
                dp_of[orders[b, c]] = np.arange(npad)
                e_dp = dp_of[ld[inb]]
                e_w, e_dpw = e_dp // P, e_dp % P
                e_q, e_i = e_dpw // SUBG, e_dpw % SUBG
                so = np.argsort(e_dp, kind="stable")
                ks = e_dp[so]
                starts = np.r_[0, np.flatnonzero(np.diff(ks)) + 1]
                sizes = np.diff(np.r_[starts, ks.size])
                cum = np.arange(ks.size) - np.repeat(starts, sizes)
                e_j = np.empty(ks.size, np.int64)
                e_j[so] = cum
                d_arr = dq[e_w, e_q, b]
                pos = OFF[e_w, e_q, b] + e_i * d_arr + e_j
                assert (pos >= 0).all() and (e_j < d_arr).all()
                stream[pos] = rel_row
            percore[c][f"idx_{tag}"] = _pack_idx(stream)
            # perm/invdeg stored [P, nbank*NW], bank-major columns
            percore[c][f"perm_{tag}"] = np.ascontiguousarray(
                perm_cols[:, c].astype(np.float32).reshape(
                    nbank * NW, P).T)
            percore[c][f"invdeg_{tag}"] = np.ascontiguousarray(
                invdeg_cols[:, c].reshape(nbank * NW, P).T)

        S["rels"].append(dict(r=r, snt=snt, dnt=dnt, tag=tag, NW=NW,
                              banks=banks, nbank=nbank, blocks=blocks,
                              gathers=gathers, nslots=nslots, maxg=maxg))

    for nt in NTYPES:
        x = np.asarray(inputs["x_" + nt])
        for c in range(ncore):
            sh = shard[nt]
            percore[c][f"xT_{nt}"] = np.ascontiguousarray(
                x[c * sh:(c + 1) * sh].T).astype(BF16)

    com = dict()
    for nt in NTYPES:
        com[f"We_{nt}"] = np.asarray(inputs["We_" + nt]).astype(BF16)
        com[f"be_{nt}"] = np.asarray(inputs["be_" + nt]).astype(
            np.float32).reshape(-1, 1)
    for l in range(3):
        com[f"W{l}"] = np.asarray(inputs[f"W{l}"]).astype(BF16)
        com[f"L{l}"] = np.asarray(inputs[f"L{l}"]).astype(BF16)
        com[f"b{l}"] = np.asarray(inputs[f"b{l}"]).astype(np.float32).reshape(-1, 1)
    com["iota"] = np.tile(np.arange(P, dtype=np.float32), (P, 1))
    for c in range(ncore):
        percore[c].update(com)
    return S, percore


# ---------------------------------------------------------------------------
# device program
# ---------------------------------------------------------------------------

def build(S):
    cfg = S["cfg"]
    ncore = cfg["NCORE"]
    nw, shard = S["nw"], S["shard"]
    DH, DOUT = cfg["D_H"], cfg["D_OUT"]
    NREL = len(cfg["RELS"])
    nsh_tot = sum(shard.values())
    maxg_all = max(R["maxg"] for R in S["rels"])
    maxw_cols = max(nw[nt] for nt in NTYPES) * P

    nc = bacc.Bacc("TRN2", target_bir_lowering=False, debug=False,
                   num_devices=ncore)

    par = {}
    for nt in NTYPES:
        par[f"xT_{nt}"] = nc.declare_dram_parameter(
            f"xT_{nt}", [cfg["MOD"][nt], shard[nt]], mybir.dt.bfloat16, False)
        par[f"We_{nt}"] = nc.declare_dram_parameter(
            f"We_{nt}", [cfg["MOD"][nt], cfg["D_IN"]], mybir.dt.bfloat16, False)
        par[f"be_{nt}"] = nc.declare_dram_parameter(
            f"be_{nt}", [cfg["D_IN"], 1], mybir.dt.float32, False)
    for l in range(3):
        od = DOUT if l == 2 else DH
        par[f"W{l}"] = nc.declare_dram_parameter(
            f"W{l}", [NREL, DH, od], mybir.dt.bfloat16, False)
        par[f"L{l}"] = nc.declare_dram_parameter(
            f"L{l}", [DH, od], mybir.dt.bfloat16, False)
        par[f"b{l}"] = nc.declare_dram_parameter(
            f"b{l}", [od, 1], mybir.dt.float32, False)
    par["iota"] = nc.declare_dram_parameter("iota", [P, P], mybir.dt.float32, False)
    for R in S["rels"]:
        tg = R["tag"]
        par[f"idx_{tg}"] = nc.declare_dram_parameter(
            f"idx_{tg}", [P, R["nslots"] // 16], mybir.dt.int16, False)
        par[f"perm_{tg}"] = nc.declare_dram_parameter(
            f"perm_{tg}", [P, R["nbank"] * R["NW"]], mybir.dt.float32, False)
        par[f"invdeg_{tg}"] = nc.declare_dram_parameter(
            f"invdeg_{tg}", [P, R["nbank"] * R["NW"]], mybir.dt.float32, False)
    out_par = nc.declare_dram_parameter("out", [nsh_tot, DOUT],
                                        mybir.dt.float32, True)

    agin, tabs = {}, {}
    for l in range(3):
        for nt in SRC_NTYPES:
            agin[(l, nt)] = nc.dram_tensor(
                f"agin{l}_{nt}", [shard[nt], DH], mybir.dt.bfloat16)
            tabs[(l, nt)] = nc.dram_tensor(
                f"tab{l}_{nt}", [cfg["N"][nt] + 2, DH], mybir.dt.bfloat16,
                addr_space="Shared")

    rel_by_tag = {R["tag"]: R for R in S["rels"]}

    with ExitStack() as ctx:
        tc = ctx.enter_context(tile.TileContext(nc))
        nc.gpsimd.load_library(library_config.mlp)

        const = ctx.enter_context(tc.tile_pool(name="const", bufs=1))
        persist = ctx.enter_context(tc.tile_pool(name="persist", bufs=1))
        gpool = ctx.enter_context(tc.tile_pool(name="gpool", bufs=3))
        ipool = ctx.enter_context(tc.tile_pool(name="ipool", bufs=4))
        xpool = ctx.enter_context(tc.tile_pool(name="xpool", bufs=2))
        wpool = ctx.enter_context(tc.tile_pool(name="wpool", bufs=4))
        prpool = ctx.enter_context(tc.tile_pool(name="prpool", bufs=2))
        pst = ctx.enter_context(tc.tile_pool(name="pst", bufs=2, space="PSUM"))
        ps1 = ctx.enter_context(tc.tile_pool(name="ps1", bufs=2, space="PSUM"))
        ps2 = ctx.enter_context(tc.tile_pool(name="ps2", bufs=2, space="PSUM"))
        psE = ctx.enter_context(tc.tile_pool(name="psE", bufs=2, space="PSUM"))

        sb_iota = const.tile([P, P], mybir.dt.float32)
        nc.sync.dma_start(sb_iota[:], par["iota"][:])
        identity = const.tile([P, P], mybir.dt.float32)
        from concourse.masks import make_identity
        make_identity(nc, identity[:])
        identity16 = const.tile([P, P], mybir.dt.bfloat16)
        nc.vector.tensor_copy(identity16[:], identity[:])

        sb_W, sb_L, sb_b = {}, {}, {}
        for l in range(3):
            od = DOUT if l == 2 else DH
            t = const.tile([DH, NREL, od], mybir.dt.bfloat16, tag=f"W{l}")
            nc.sync.dma_start(t[:], par[f"W{l}"][:].rearrange("r k o -> k r o"))
            sb_W[l] = t
            sb_L[l] = const.tile([DH, od], mybir.dt.bfloat16, tag=f"L{l}", name=f"L{l}")
            nc.sync.dma_start(sb_L[l][:], par[f"L{l}"][:])
            sb_b[l] = const.tile([od, 1], mybir.dt.float32, tag=f"b{l}", name=f"b{l}")
            nc.sync.dma_start(sb_b[l][:], par[f"b{l}"][:])
        sb_meta = {}
        for R in S["rels"]:
            tg = R["tag"]
            pm = const.tile([P, R["nbank"] * R["NW"]], mybir.dt.float32,
                            tag=f"pm_{tg}")
            nc.sync.dma_start(pm[:], par[f"perm_{tg}"][:])
            iv = const.tile([P, R["nbank"] * R["NW"]], mybir.dt.float32,
                            tag=f"iv_{tg}")
            nc.sync.dma_start(iv[:], par[f"invdeg_{tg}"][:])
            sb_meta[tg] = (pm, iv)

        zrow = const.tile([1, DH], mybir.dt.bfloat16)
        nc.vector.memset(zrow[:], 0.0)
        for l in range(3):
            for nt in SRC_NTYPES:
                n = cfg["N"][nt]
                nc.sync.dma_start(tabs[(l, nt)][0:1, :], zrow[:])
                nc.sync.dma_start(tabs[(l, nt)][n + 1:n + 2, :], zrow[:])

        hT = [persist.tile([DH, nsh_tot], mybir.dt.bfloat16, tag=f"hT{i}",
                           name=f"hT{i}")
              for i in range(2)]
        nt_off, o = {}, 0
        for nt in NTYPES:
            nt_off[nt] = o
            o += shard[nt]
        agg = persist.tile([DH, nsh_tot], mybir.dt.float32, tag="agg")

        def emit_ag(l, nt):
            """Transpose this core's h shard of ntype nt and AllGather it
            into the layer-l gather table."""
            sh = shard[nt]
            for w0 in range(0, sh, P):
                cols = min(P, sh - w0)
                src = hT[l % 2][:, nt_off[nt] + w0:nt_off[nt] + w0 + cols]
                pt = pst.tile([P, P], mybir.dt.bfloat16, tag="tp", name="pt16")
                nc.tensor.transpose(pt[:cols, :DH], src, identity16[:])
                stg = wpool.tile([P, DH], mybir.dt.bfloat16, tag="agstg")
                nc.vector.tensor_copy(stg[:cols, :], pt[:cols, :DH])
                nc.sync.dma_start(agin[(l, nt)][w0:w0 + cols, :],
                                  stg[:cols, :])
            nc.gpsimd.collective_compute(
                "AllGather", mybir.AluOpType.bypass,
                replica_groups=[list(range(ncore))],
                ins=[agin[(l, nt)][:]],
                outs=[tabs[(l, nt)][1:cfg["N"][nt] + 1]],
            )

        def emit_embedding():
            for nt in ("drug", "gene", "disease"):
                mod, sh = cfg["MOD"][nt], shard[nt]
                kt = mod // P
                sb_we = xpool.tile([P, 8, cfg["D_IN"]], mybir.dt.bfloat16, tag="we")
                nc.sync.dma_start(
                    sb_we[:, :kt, :],
                    par[f"We_{nt}"][:].rearrange("(k p) f -> p k f", p=P))
                sb_be = wpool.tile([cfg["D_IN"], 1], mybir.dt.float32, tag="be")
                nc.sync.dma_start(sb_be[:], par[f"be_{nt}"][:])
                for n0 in range(0, sh, 512):
                    n1 = min(n0 + 512, sh)
                    cols = n1 - n0
                    xt = xpool.tile([P, 8, 512], mybir.dt.bfloat16, tag="xt")
                    nc.sync.dma_start(
                        xt[:, :kt, :cols],
                        par[f"xT_{nt}"][:].rearrange(
                            "(k p) n -> p k n", p=P)[:, :, n0:n1])
                    pe = psE.tile([P, 512], mybir.dt.float32, tag="emb")
                    for k in range(kt):
                        nc.tensor.matmul(pe[:, :cols], sb_we[:, k, :],
                                         xt[:, k, :cols],
                                         start=(k == 0), stop=(k == kt - 1))
                    nc.scalar.activation(
                        hT[0][:, nt_off[nt] + n0:nt_off[nt] + n1], pe[:, :cols],
                        mybir.ActivationFunctionType.Identity, bias=sb_be[:])
                if nt in SRC_NTYPES:
                    emit_ag(0, nt)

        def emit_relation(l, R):
            """Per bank: gathers + segment sums into praw, then per-window
            W_r + unpermute*invdeg matmuls accumulated into agg columns of
            R's dst ntype (bank contributions sum in agg)."""
            od = DOUT if l == 2 else DH
            tg, snt, dnt, r, NW = R["tag"], R["snt"], R["dnt"], R["r"], R["NW"]
            tab = tabs[(l, snt)]
            pm, iv = sb_meta[tg]
            blk_by_g = {gi: [] for gi in range(len(R["gathers"]))}
            for blk in R["blocks"]:
                for gi, (gb, goff, gslots) in enumerate(R["gathers"]):
                    if gb == blk[1] and goff <= blk[4] < goff + gslots:
                        blk_by_g[gi].append(blk)
                        break
            for b, (b0, b1) in enumerate(R["banks"]):
                praw = prpool.tile([P, maxw_cols], mybir.dt.bfloat16,
                                   tag="praw")
                written = np.zeros((NW, NSUB), bool)
                for gi, (gb, goff, gslots) in enumerate(R["gathers"]):
                    if gb != b:
                        continue
                    sbi = ipool.tile([P, maxg_all // 16], mybir.dt.int16,
                                     tag="idx")
                    nc.sync.dma_start(
                        sbi[:, :gslots // 16],
                        par[f"idx_{tg}"][:, goff // 16:(goff + gslots) // 16])
                    gt = gpool.tile([P, 1, maxg_all], mybir.dt.bfloat16,
                                    tag="gat")
                    nc.gpsimd.dma_gather(
                        out_ap=gt[:, :, :gslots], in_ap=tab[b0:b1],
                        idxs_ap=sbi[:, :gslots // 16],
                        num_idxs=gslots, num_idxs_reg=gslots,
                        elem_size=DH, transpose=True,
                        single_packet=(gslots <= 992))
                    for (w, bb, q, d, off) in blk_by_g[gi]:
                        loc = off - goff
                        view = gt[:, 0, loc:loc + d * SUBG].rearrange(
                            "p (n d) -> p n d", d=d)
                        cols = slice(w * P + q * SUBG, w * P + (q + 1) * SUBG)
                        with nc.allow_low_precision(
                                reason="DVE reduces in fp32; bf16 rounding "
                                       "applies once at output"):
                            nc.vector.tensor_reduce(
                                praw[:, cols], view, axis=mybir.AxisListType.X,
                                op=mybir.AluOpType.add)
                        written[w, q] = True
                for w in range(NW):
                    for q in range(NSUB):
                        if not written[w, q]:
                            nc.vector.memset(
                                praw[:, w * P + q * SUBG:
                                     w * P + (q + 1) * SUBG], 0.0)
                for w in range(NW):
                    wb = b * NW + w
                    Sp = wpool.tile([P, P], mybir.dt.bfloat16, tag="Sperm")
                    nc.vector.tensor_scalar(
                        Sp[:], sb_iota[:], pm[:, wb:wb + 1], iv[:, wb:wb + 1],
                        op0=mybir.AluOpType.is_equal, op1=mybir.AluOpType.mult)
                    p1 = ps1.tile([P, DH], mybir.dt.float32, tag="out1")
                    nc.tensor.matmul(p1[:, :od], praw[:, w * P:(w + 1) * P],
                                     sb_W[l][:, r, :],
                                     start=True, stop=True)
                    o1 = wpool.tile([P, DH], mybir.dt.bfloat16, tag="o1")
                    nc.vector.tensor_copy(o1[:, :od], p1[:, :od])
                    p2 = ps2.tile([P, P], mybir.dt.float32, tag="out2")
                    nc.tensor.matmul(p2[:od, :], o1[:, :od], Sp[:],
                                     start=True, stop=True)
                    cs = nt_off[dnt] + w * P
                    ce = min(cs + P, nt_off[dnt] + shard[dnt])
                    nc.vector.tensor_add(agg[:od, cs:ce], agg[:od, cs:ce],
                                         p2[:od, :ce - cs])

        def emit_finish_ntype(l, nt):
            """Self-loop + bias (+relu) for ntype nt; write hT (or output)."""
            od = DOUT if l == 2 else DH
            sh = shard[nt]
            for w0 in range(0, sh, P):
                cols = min(P, sh - w0)
                cs = nt_off[nt] + w0
                p2 = ps2.tile([P, P], mybir.dt.float32, tag="out2")
                nc.tensor.matmul(p2[:od, :cols], sb_L[l][:],
                                 hT[l % 2][:, cs:cs + cols],
                                 start=True, stop=True)
                nc.vector.tensor_add(agg[:od, cs:cs + cols],
                                     agg[:od, cs:cs + cols],
                                     p2[:od, :cols])
                if l < 2:
                    nc.scalar.activation(
                        hT[(l + 1) % 2][:od, cs:cs + cols],
                        agg[:od, cs:cs + cols],
                        mybir.ActivationFunctionType.Relu, bias=sb_b[l][:])
                else:
                    fin = wpool.tile([P, P], mybir.dt.float32, tag="fin")
                    nc.scalar.activation(
                        fin[:od, :cols], agg[:od, cs:cs + cols],
                        mybir.ActivationFunctionType.Identity,
                        bias=sb_b[l][:])
                    pt = pst.tile([P, P], mybir.dt.float32, tag="tp")
                    nc.tensor.transpose(pt[:cols, :od], fin[:od, :cols],
                                        identity[:od, :od])
                    stg = wpool.tile([P, DOUT], mybir.dt.float32, tag="ostg")
                    nc.vector.tensor_copy(stg[:cols, :], pt[:cols, :od])
                    nc.sync.dma_start(out_par[cs:cs + cols, :],
                                      stg[:cols, :])
            if l < 2 and nt in SRC_NTYPES:
                emit_ag(l + 1, nt)

        def emit_layer(l):
            od = DOUT if l == 2 else DH
            # first relation targeting each dst ntype clears its agg columns
            cleared = set()
            # ntype completed when all relations targeting it are done
            remaining = {nt: sum(1 for R in S["rels"] if R["dnt"] == nt)
                         for nt in NTYPES}
            for tg in REL_ORDER:
                R = rel_by_tag[tg]
                dnt = R["dnt"]
                if dnt not in cleared:
                    cs, sh = nt_off[dnt], shard[dnt]
                    nc.vector.memset(agg[:od, cs:cs + sh], 0.0)
                    cleared.add(dnt)
                emit_relation(l, R)
                remaining[dnt] -= 1
                if remaining[dnt] == 0:
                    emit_finish_ntype(l, dnt)

        emit_embedding()
        emit_layer(0)
        emit_layer(1)
        emit_layer(2)

    nc.compile()
    return nc


# ---------------------------------------------------------------------------
# entry point
# ---------------------------------------------------------------------------

def _install_ntff_hook():
    if "antenv.axon_hooks" in sys.modules:
        return
    mod = types.ModuleType("antenv.axon_hooks")
    mod._hook = None
    mod.set_axon_ntff_profile_hook = lambda h: setattr(mod, "_hook", h)
    mod.get_axon_ntff_profile_hook = lambda: mod._hook
    sys.modules["antenv.axon_hooks"] = mod
    try:
        import antenv
        antenv.axon_hooks = mod
        from trn_agent_boot.trn_boot import _ntff_profile_via_ctypes
        hook = _ntff_profile_via_ctypes("/opt/axon/libaxon_pjrt.so")
        if hook is not None:
            mod.set_axon_ntff_profile_hook(hook)
    except Exception:
        pass


def run(inputs, cfg=CFG, trace=False, tmpdir=None):
    S, percore = preprocess(cfg, inputs)
    nc = build(S)
    _install_ntff_hook()
    from concourse import bass_utils
    bass_utils.upload_artifacts = lambda d: d
    res = bass_utils.run_bass_kernel_spmd(
        nc, percore, list(range(cfg["NCORE"])), trace=trace, tmpdir=tmpdir,
        trace_cores=[0] if trace else None)
    ncore = cfg["NCORE"]
    shard = {nt: cfg["N"][nt] // ncore for nt in NTYPES}
    outs = []
    o = 0
    for nt in NTYPES:
        parts = [res.results[c]["out"][o:o + shard[nt]] for c in range(ncore)]
        outs.append(np.concatenate(parts, 0))
        o += shard[nt]
    full = np.concatenate(outs, 0).astype(np.float32)
    run.last_exec_time_ns = res.exec_time_ns
    return full


def kernel(**inputs):
    return run(inputs)


# revision 17
# speedup vs baseline: 1.0070x; 1.0070x over previous
"""Trainium2 Bass kernel for nn_BaseRGCNHetero (3-layer heterogeneous RGCN).

Strategy (8 NeuronCores, SPMD):
  - Destination-shard the nodes: core c owns rows [c*N/8, (c+1)*N/8) of every
    node type; all edges whose dst is in the shard are processed there, so
    per-relation aggregates need no cross-core reduction.
  - Aggregate-first algebra: agg[dst] = (sum_{e->dst} h[src]) @ W_r * inv_deg,
    sharing one bf16 gather table per source ntype (drug, gene) per layer.
  - After each layer the drug/gene h-shards are AllGathered (bf16) into
    per-core DRAM gather tables for the next layer.  Relations are ordered
    ddr -> dg -> gg -> dd -> gd so each ntype's activation + AllGather fires
    as early as possible and overlaps the remaining relations' gathers.
  - Segment sums: host lays edges out as a padded, degree-bucketed slot
    stream per (relation, index bank, 128-dst window, 8-dst subgroup).
    dma_gather (bf16, transpose=True) produces feature-major tiles; VectorE
    tensor_reduce over the innermost (slot) axis yields segment sums.  Pad
    slots point at an all-zero table row.  Tables over 32768 rows split into
    two int16 index banks; each bank gets its own per-window degree sort so
    subgroup depth padding stays low.
  - Per (relation, bank, window): a one-hot "unpermute * inv_deg" matrix is
    built by a fused tensor_scalar(is_equal, mult); two matmuls apply W_r and
    the window permutation back to natural dst order, accumulating into a
    feature-major fp32 SBUF accumulator (bank contributions sum there); the
    self-loop h @ L is one more matmul; bias+relu is a fused ScalarE
    activation per window.
"""
import sys
import types
import numpy as np
import ml_dtypes
from contextlib import ExitStack

import concourse.bass as bass
import concourse.bacc as bacc
import concourse.tile as tile
from concourse import mybir, library_config

BF16 = ml_dtypes.bfloat16
P = 128
SUBG = 16          # dsts per reduce subgroup
NSUB = P // SUBG   # subgroups per window
GCAP = 5120        # target max slots per dma_gather

CFG = dict(
    N={"drug": 20000, "gene": 50000, "disease": 10000},
    MOD={"drug": 1024, "gene": 768, "disease": 512},
    D_IN=128, D_H=128, D_OUT=64,
    RELS=[("drug", "disease", "dd"), ("drug", "drug", "ddr"),
          ("drug", "gene", "dg"), ("gene", "disease", "gd"),
          ("gene", "gene", "gg")],
    NCORE=8,
)

NTYPES = ("drug", "gene", "disease")
SRC_NTYPES = ("drug", "gene")
# processing order inside a layer: drug-dst first (unlocks drug AllGather),
# then gene-dst, then disease-dst (no AllGather needed).
REL_ORDER = ["ddr", "dg", "gg", "dd", "gd"]


# ---------------------------------------------------------------------------
# host-side preprocessing
# ---------------------------------------------------------------------------

def _pack_idx(stream):
    """int array (len % 128 == 0) -> dma_gather idx layout [128, len/16] int16:
    idx i at (i%16, i//16), replicated across the 8 groups of 16 partitions."""
    n = stream.size
    v = stream.astype(np.int16).reshape(n // 16, 16).T
    return np.tile(v, (8, 1))


def preprocess(cfg, inputs):
    ncore = cfg["NCORE"]
    shard = {nt: cfg["N"][nt] // ncore for nt in NTYPES}
    nw = {nt: -(-shard[nt] // P) for nt in NTYPES}

    S = dict(cfg=cfg, nw=nw, shard=shard, rels=[])
    percore = [dict() for _ in range(ncore)]

    BANK = 32768
    for r, (snt, dnt, tag) in enumerate(cfg["RELS"]):
        src = np.asarray(inputs["e_" + tag + "_s"]).astype(np.int64)
        dst = np.asarray(inputs["e_" + tag + "_d"]).astype(np.int64)
        trows = cfg["N"][snt] + 2
        # table row 0 is all-zero (pad target); banks split the int16 idx range
        if trows <= BANK:
            banks = [(0, trows)]
        else:
            banks = [(0, BANK), (BANK - 1, trows)]  # overlap row BANK-1 unused;
            # bank1 pad slots use relative row 0 -> absolute BANK-1 (a real
            # node!), so bank1 pads instead point at relative (trows-1-b0)
            # which is the trailing all-zero row.
        nbank = len(banks)
        NW = nw[dnt]
        dsh = shard[dnt]
        npad = NW * P

        core_of = dst // dsh
        deg_all = np.bincount(dst, minlength=cfg["N"][dnt]).astype(np.int64)
        row_all = src + 1
        bank_of = np.zeros(src.size, np.int64)
        for b, (b0, b1) in enumerate(banks):
            bank_of[(row_all >= b0 + (1 if b else 0)) & (row_all < b1)] = b
        pad_rel = [(trows - 1) - b0 for b0, b1 in banks]  # trailing zero row
        pad_rel[0] = 0                                    # leading zero row

        # per-bank, per-core window-local orderings by per-bank degree
        orders = np.zeros((nbank, ncore, npad), np.int64)
        perm_cols = np.zeros((nbank, ncore, NW, P), np.int32)
        invdeg_cols = np.zeros((nbank, ncore, NW, P), np.float32)
        dcnt = np.zeros((nbank, ncore, NW, P), np.int64)
        dn = np.arange(npad)
        for c in range(ncore):
            m = core_of == c
            ld_all = dst[m] - c * dsh
            deg_pad = np.zeros(npad, np.int64)
            deg_pad[:dsh] = deg_all[c * dsh:(c + 1) * dsh]
            ivfull = (1.0 / np.maximum(deg_pad, 1.0)).astype(np.float32)
            ivfull[dsh:] = 0.0
            for b in range(nbank):
                cb = np.bincount(ld_all[bank_of[m] == b], minlength=npad)
                order = np.lexsort((dn, -cb, dn // P))
                orders[b, c] = order
                perm_cols[b, c] = (order % P).reshape(NW, P)
                iv = ivfull[order].copy()
                iv[cb[order] == 0] = 0.0   # no bank-b edges -> contribute 0
                invdeg_cols[b, c] = iv.reshape(NW, P)
                dcnt[b, c] = cb[order].reshape(NW, P)

        # subgroup depths, common across cores; window block sizes % 128
        dq = np.zeros((NW, NSUB, nbank), np.int64)
        for q in range(NSUB):
            dq[:, q, :] = dcnt[:, :, :, q * SUBG:(q + 1) * SUBG].max(
                axis=(1, 3)).T
        for b in range(nbank):
            dq[:, NSUB - 1, b] += (-dq[:, :, b].sum(axis=1)) % (P // SUBG)

        # block layout: bank-major, then window, subgroup; greedy gathers
        blocks = []
        OFF = np.full((NW, NSUB, nbank), -1, np.int64)
        off = 0
        gathers = []
        for b in range(nbank):
            gstart, gslots = off, 0
            for w in range(NW):
                wslots = int(dq[w, :, b].sum()) * SUBG
                if wslots == 0:
                    continue
                if gslots + wslots > GCAP and gslots > 0:
                    gathers.append((b, gstart, gslots))
                    gstart, gslots = off, 0
                for q in range(NSUB):
                    if dq[w, q, b] > 0:
                        blocks.append((w, b, q, int(dq[w, q, b]), off))
                        OFF[w, q, b] = off
                        off += int(dq[w, q, b]) * SUBG
                gslots += wslots
            if gslots > 0:
                gathers.append((b, gstart, gslots))
        nslots = max(off, P)
        maxg = max((g[2] for g in gathers), default=P)

        for c in range(ncore):
            stream = np.zeros(nslots, np.int64)
            for (w, b, q, d, o) in blocks:
                stream[o:o + d * SUBG] = pad_rel[b]
            m = core_of == c
            sm_row = row_all[m]
            sm_bank = bank_of[m]
            ld = dst[m] - c * dsh
            for b, (b0, b1) in enumerate(banks):
                inb = sm_bank == b
                if not inb.any():
                    continue
                rel_row = sm_row[inb] - b0
                dp_of = np.zeros(npad, np.int64)
                dp_of[orders[b, c]] = np.arange(npad)
                e_dp = dp_of[ld[inb]]
                e_w, e_dpw = e_dp // P, e_dp % P
                e_q, e_i = e_dpw // SUBG, e_dpw % SUBG
                so = np.argsort(e_dp, kind="stable")
                ks = e_dp[so]
                starts = np.r_[0, np.flatnonzero(np.diff(ks)) + 1]
                sizes = np.diff(np.r_[starts, ks.size])
                cum = np.arange(ks.size) - np.repeat(starts, sizes)
                e_j = np.empty(ks.size, np.int64)
                e_j[so] = cum
                d_arr = dq[e_w, e_q, b]
                pos = OFF[e_w, e_q, b] + e_i * d_arr + e_j
                assert (pos >= 0).all() and (e_j < d_arr).all()
                stream[pos] = rel_row
            percore[c][f"idx_{tag}"] = _pack_idx(stream)
            # perm/invdeg stored [P, nbank*NW], bank-major columns
            percore[c][f"perm_{tag}"] = np.ascontiguousarray(
                perm_cols[:, c].astype(np.float32).reshape(
                    nbank * NW, P).T)
            percore[c][f"invdeg_{tag}"] = np.ascontiguousarray(
                invdeg_cols[:, c].reshape(nbank * NW, P).T)

        S["rels"].append(dict(r=r, snt=snt, dnt=dnt, tag=tag, NW=NW,
                              banks=banks, nbank=nbank, blocks=blocks,
                              gathers=gathers, nslots=nslots, maxg=maxg))

    for nt in NTYPES:
        x = np.asarray(inputs["x_" + nt])
        for c in range(ncore):
            sh = shard[nt]
            percore[c][f"xT_{nt}"] = np.ascontiguousarray(
                x[c * sh:(c + 1) * sh].T).astype(BF16)

    com = dict()
    for nt in NTYPES:
        com[f"We_{nt}"] = np.asarray(inputs["We_" + nt]).astype(BF16)
        com[f"be_{nt}"] = np.asarray(inputs["be_" + nt]).astype(
            np.float32).reshape(-1, 1)
    for l in range(3):
        com[f"W{l}"] = np.asarray(inputs[f"W{l}"]).astype(BF16)
        com[f"L{l}"] = np.asarray(inputs[f"L{l}"]).astype(BF16)
        com[f"b{l}"] = np.asarray(inputs[f"b{l}"]).astype(np.float32).reshape(-1, 1)
    com["iota"] = np.tile(np.arange(P, dtype=np.float32), (P, 1))
    for c in range(ncore):
        percore[c].update(com)
    return S, percore


# ---------------------------------------------------------------------------
# device program
# ---------------------------------------------------------------------------

def build(S):
    cfg = S["cfg"]
    ncore = cfg["NCORE"]
    nw, shard = S["nw"], S["shard"]
    DH, DOUT = cfg["D_H"], cfg["D_OUT"]
    NREL = len(cfg["RELS"])
    nsh_tot = sum(shard.values())
    maxg_all = max(R["maxg"] for R in S["rels"])
    maxw_cols = max(nw[nt] for nt in NTYPES) * P

    nc = bacc.Bacc("TRN2", target_bir_lowering=False, debug=False,
                   num_devices=ncore, dynamic_dma_scratch_size=32768)

    par = {}
    for nt in NTYPES:
        par[f"xT_{nt}"] = nc.declare_dram_parameter(
            f"xT_{nt}", [cfg["MOD"][nt], shard[nt]], mybir.dt.bfloat16, False)
        par[f"We_{nt}"] = nc.declare_dram_parameter(
            f"We_{nt}", [cfg["MOD"][nt], cfg["D_IN"]], mybir.dt.bfloat16, False)
        par[f"be_{nt}"] = nc.declare_dram_parameter(
            f"be_{nt}", [cfg["D_IN"], 1], mybir.dt.float32, False)
    for l in range(3):
        od = DOUT if l == 2 else DH
        par[f"W{l}"] = nc.declare_dram_parameter(
            f"W{l}", [NREL, DH, od], mybir.dt.bfloat16, False)
        par[f"L{l}"] = nc.declare_dram_parameter(
            f"L{l}", [DH, od], mybir.dt.bfloat16, False)
        par[f"b{l}"] = nc.declare_dram_parameter(
            f"b{l}", [od, 1], mybir.dt.float32, False)
    par["iota"] = nc.declare_dram_parameter("iota", [P, P], mybir.dt.float32, False)
    for R in S["rels"]:
        tg = R["tag"]
        par[f"idx_{tg}"] = nc.declare_dram_parameter(
            f"idx_{tg}", [P, R["nslots"] // 16], mybir.dt.int16, False)
        par[f"perm_{tg}"] = nc.declare_dram_parameter(
            f"perm_{tg}", [P, R["nbank"] * R["NW"]], mybir.dt.float32, False)
        par[f"invdeg_{tg}"] = nc.declare_dram_parameter(
            f"invdeg_{tg}", [P, R["nbank"] * R["NW"]], mybir.dt.float32, False)
    out_par = nc.declare_dram_parameter("out", [nsh_tot, DOUT],
                                        mybir.dt.float32, True)

    agin, tabs = {}, {}
    for l in range(3):
        for nt in SRC_NTYPES:
            agin[(l, nt)] = nc.dram_tensor(
                f"agin{l}_{nt}", [shard[nt], DH], mybir.dt.bfloat16)
            tabs[(l, nt)] = nc.dram_tensor(
                f"tab{l}_{nt}", [cfg["N"][nt] + 2, DH], mybir.dt.bfloat16,
                addr_space="Shared")

    rel_by_tag = {R["tag"]: R for R in S["rels"]}

    with ExitStack() as ctx:
        tc = ctx.enter_context(tile.TileContext(nc))
        nc.gpsimd.load_library(library_config.mlp)

        const = ctx.enter_context(tc.tile_pool(name="const", bufs=1))
        persist = ctx.enter_context(tc.tile_pool(name="persist", bufs=1))
        gpool = ctx.enter_context(tc.tile_pool(name="gpool", bufs=3))
        ipool = ctx.enter_context(tc.tile_pool(name="ipool", bufs=4))
        xpool = ctx.enter_context(tc.tile_pool(name="xpool", bufs=2))
        wpool = ctx.enter_context(tc.tile_pool(name="wpool", bufs=4))
        prpool = ctx.enter_context(tc.tile_pool(name="prpool", bufs=2))
        pst = ctx.enter_context(tc.tile_pool(name="pst", bufs=2, space="PSUM"))
        ps1 = ctx.enter_context(tc.tile_pool(name="ps1", bufs=2, space="PSUM"))
        ps2 = ctx.enter_context(tc.tile_pool(name="ps2", bufs=2, space="PSUM"))
        psE = ctx.enter_context(tc.tile_pool(name="psE", bufs=2, space="PSUM"))

        sb_iota = const.tile([P, P], mybir.dt.float32)
        nc.sync.dma_start(sb_iota[:], par["iota"][:])
        identity = const.tile([P, P], mybir.dt.float32)
        from concourse.masks import make_identity
        make_identity(nc, identity[:])
        identity16 = const.tile([P, P], mybir.dt.bfloat16)
        nc.vector.tensor_copy(identity16[:], identity[:])

        sb_W, sb_L, sb_b = {}, {}, {}
        for l in range(3):
            od = DOUT if l == 2 else DH
            t = const.tile([DH, NREL, od], mybir.dt.bfloat16, tag=f"W{l}")
            nc.sync.dma_start(t[:], par[f"W{l}"][:].rearrange("r k o -> k r o"))
            sb_W[l] = t
            sb_L[l] = const.tile([DH, od], mybir.dt.bfloat16, tag=f"L{l}", name=f"L{l}")
            nc.sync.dma_start(sb_L[l][:], par[f"L{l}"][:])
            sb_b[l] = const.tile([od, 1], mybir.dt.float32, tag=f"b{l}", name=f"b{l}")
            nc.sync.dma_start(sb_b[l][:], par[f"b{l}"][:])
        sb_meta = {}
        for R in S["rels"]:
            tg = R["tag"]
            pm = const.tile([P, R["nbank"] * R["NW"]], mybir.dt.float32,
                            tag=f"pm_{tg}")
            nc.sync.dma_start(pm[:], par[f"perm_{tg}"][:])
            iv = const.tile([P, R["nbank"] * R["NW"]], mybir.dt.float32,
                            tag=f"iv_{tg}")
            nc.sync.dma_start(iv[:], par[f"invdeg_{tg}"][:])
            sb_meta[tg] = (pm, iv)

        zrow = const.tile([1, DH], mybir.dt.bfloat16)
        nc.vector.memset(zrow[:], 0.0)
        for l in range(3):
            for nt in SRC_NTYPES:
                n = cfg["N"][nt]
                nc.sync.dma_start(tabs[(l, nt)][0:1, :], zrow[:])
                nc.sync.dma_start(tabs[(l, nt)][n + 1:n + 2, :], zrow[:])

        hT = [persist.tile([DH, nsh_tot], mybir.dt.bfloat16, tag=f"hT{i}",
                           name=f"hT{i}")
              for i in range(2)]
        nt_off, o = {}, 0
        for nt in NTYPES:
            nt_off[nt] = o
            o += shard[nt]
        agg = persist.tile([DH, nsh_tot], mybir.dt.float32, tag="agg")

        def emit_ag(l, nt):
            """Transpose this core's h shard of ntype nt and AllGather it
            into the layer-l gather table."""
            sh = shard[nt]
            for w0 in range(0, sh, P):
                cols = min(P, sh - w0)
                src = hT[l % 2][:, nt_off[nt] + w0:nt_off[nt] + w0 + cols]
                pt = pst.tile([P, P], mybir.dt.bfloat16, tag="tp", name="pt16")
                nc.tensor.transpose(pt[:cols, :DH], src, identity16[:])
                stg = wpool.tile([P, DH], mybir.dt.bfloat16, tag="agstg")
                nc.vector.tensor_copy(stg[:cols, :], pt[:cols, :DH])
                nc.sync.dma_start(agin[(l, nt)][w0:w0 + cols, :],
                                  stg[:cols, :])
            nc.gpsimd.collective_compute(
                "AllGather", mybir.AluOpType.bypass,
                replica_groups=[list(range(ncore))],
                ins=[agin[(l, nt)][:]],
                outs=[tabs[(l, nt)][1:cfg["N"][nt] + 1]],
            )

        def emit_embedding():
            for nt in ("drug", "gene", "disease"):
                mod, sh = cfg["MOD"][nt], shard[nt]
                kt = mod // P
                sb_we = xpool.tile([P, 8, cfg["D_IN"]], mybir.dt.bfloat16, tag="we")
                nc.sync.dma_start(
                    sb_we[:, :kt, :],
                    par[f"We_{nt}"][:].rearrange("(k p) f -> p k f", p=P))
                sb_be = wpool.tile([cfg["D_IN"], 1], mybir.dt.float32, tag="be")
                nc.sync.dma_start(sb_be[:], par[f"be_{nt}"][:])
                for n0 in range(0, sh, 512):
                    n1 = min(n0 + 512, sh)
                    cols = n1 - n0
                    xt = xpool.tile([P, 8, 512], mybir.dt.bfloat16, tag="xt")
                    nc.sync.dma_start(
                        xt[:, :kt, :cols],
                        par[f"xT_{nt}"][:].rearrange(
                            "(k p) n -> p k n", p=P)[:, :, n0:n1])
                    pe = psE.tile([P, 512], mybir.dt.float32, tag="emb")
                    for k in range(kt):
                        nc.tensor.matmul(pe[:, :cols], sb_we[:, k, :],
                                         xt[:, k, :cols],
                                         start=(k == 0), stop=(k == kt - 1))
                    nc.scalar.activation(
                        hT[0][:, nt_off[nt] + n0:nt_off[nt] + n1], pe[:, :cols],
                        mybir.ActivationFunctionType.Identity, bias=sb_be[:])
                if nt in SRC_NTYPES:
                    emit_ag(0, nt)

        def emit_relation(l, R):
            """Per bank: gathers + segment sums into praw, then per-window
            W_r + unpermute*invdeg matmuls accumulated into agg columns of
            R's dst ntype (bank contributions sum in agg)."""
            od = DOUT if l == 2 else DH
            tg, snt, dnt, r, NW = R["tag"], R["snt"], R["dnt"], R["r"], R["NW"]
            tab = tabs[(l, snt)]
            pm, iv = sb_meta[tg]
            blk_by_g = {gi: [] for gi in range(len(R["gathers"]))}
            for blk in R["blocks"]:
                for gi, (gb, goff, gslots) in enumerate(R["gathers"]):
                    if gb == blk[1] and goff <= blk[4] < goff + gslots:
                        blk_by_g[gi].append(blk)
                        break
            for b, (b0, b1) in enumerate(R["banks"]):
                praw = prpool.tile([P, maxw_cols], mybir.dt.bfloat16,
                                   tag="praw")
                written = np.zeros((NW, NSUB), bool)
                for gi, (gb, goff, gslots) in enumerate(R["gathers"]):
                    if gb != b:
                        continue
                    sbi = ipool.tile([P, maxg_all // 16], mybir.dt.int16,
                                     tag="idx")
                    nc.sync.dma_start(
                        sbi[:, :gslots // 16],
                        par[f"idx_{tg}"][:, goff // 16:(goff + gslots) // 16])
                    gt = gpool.tile([P, 1, maxg_all], mybir.dt.bfloat16,
                                    tag="gat")
                    nc.gpsimd.dma_gather(
                        out_ap=gt[:, :, :gslots], in_ap=tab[b0:b1],
                        idxs_ap=sbi[:, :gslots // 16],
                        num_idxs=gslots, num_idxs_reg=gslots,
                        elem_size=DH, transpose=True,
                        single_packet=(gslots <= 992))
                    for (w, bb, q, d, off) in blk_by_g[gi]:
                        loc = off - goff
                        view = gt[:, 0, loc:loc + d * SUBG].rearrange(
                            "p (n d) -> p n d", d=d)
                        cols = slice(w * P + q * SUBG, w * P + (q + 1) * SUBG)
                        with nc.allow_low_precision(
                                reason="DVE reduces in fp32; bf16 rounding "
                                       "applies once at output"):
                            nc.vector.tensor_reduce(
                                praw[:, cols], view, axis=mybir.AxisListType.X,
                                op=mybir.AluOpType.add)
                        written[w, q] = True
                for w in range(NW):
                    for q in range(NSUB):
                        if not written[w, q]:
                            nc.vector.memset(
                                praw[:, w * P + q * SUBG:
                                     w * P + (q + 1) * SUBG], 0.0)
                for w in range(NW):
                    wb = b * NW + w
                    Sp = wpool.tile([P, P], mybir.dt.bfloat16, tag="Sperm")
                    nc.vector.tensor_scalar(
                        Sp[:], sb_iota[:], pm[:, wb:wb + 1], iv[:, wb:wb + 1],
                        op0=mybir.AluOpType.is_equal, op1=mybir.AluOpType.mult)
                    p1 = ps1.tile([P, DH], mybir.dt.float32, tag="out1")
                    nc.tensor.matmul(p1[:, :od], praw[:, w * P:(w + 1) * P],
                                     sb_W[l][:, r, :],
                                     start=True, stop=True)
                    o1 = wpool.tile([P, DH], mybir.dt.bfloat16, tag="o1")
                    nc.vector.tensor_copy(o1[:, :od], p1[:, :od])
                    p2 = ps2.tile([P, P], mybir.dt.float32, tag="out2")
                    nc.tensor.matmul(p2[:od, :], o1[:, :od], Sp[:],
                                     start=True, stop=True)
                    cs = nt_off[dnt] + w * P
                    ce = min(cs + P, nt_off[dnt] + shard[dnt])
                    nc.vector.tensor_add(agg[:od, cs:ce], agg[:od, cs:ce],
                                         p2[:od, :ce - cs])

        def emit_finish_ntype(l, nt):
            """Self-loop + bias (+relu) for ntype nt; write hT (or output)."""
            od = DOUT if l == 2 else DH
            sh = shard[nt]
            for w0 in range(0, sh, P):
                cols = min(P, sh - w0)
                cs = nt_off[nt] + w0
                p2 = ps2.tile([P, P], mybir.dt.float32, tag="out2")
                nc.tensor.matmul(p2[:od, :cols], sb_L[l][:],
                                 hT[l % 2][:, cs:cs + cols],
                                 start=True, stop=True)
                nc.vector.tensor_add(agg[:od, cs:cs + cols],
                                     agg[:od, cs:cs + cols],
                                     p2[:od, :cols])
                if l < 2:
                    nc.scalar.activation(
                        hT[(l + 1) % 2][:od, cs:cs + cols],
                        agg[:od, cs:cs + cols],
                        mybir.ActivationFunctionType.Relu, bias=sb_b[l][:])
                else:
                    fin = wpool.tile([P, P], mybir.dt.float32, tag="fin")
                    nc.scalar.activation(
                        fin[:od, :cols], agg[:od, cs:cs + cols],
                        mybir.ActivationFunctionType.Identity,
                        bias=sb_b[l][:])
                    pt = pst.tile([P, P], mybir.dt.float32, tag="tp")
                    nc.tensor.transpose(pt[:cols, :od], fin[:od, :cols],
                                        identity[:od, :od])
                    stg = wpool.tile([P, DOUT], mybir.dt.float32, tag="ostg")
                    nc.vector.tensor_copy(stg[:cols, :], pt[:cols, :od])
                    nc.sync.dma_start(out_par[cs:cs + cols, :],
                                      stg[:cols, :])
            if l < 2 and nt in SRC_NTYPES:
                emit_ag(l + 1, nt)

        def emit_layer(l):
            od = DOUT if l == 2 else DH
            # first relation targeting each dst ntype clears its agg columns
            cleared = set()
            # ntype completed when all relations targeting it are done
            remaining = {nt: sum(1 for R in S["rels"] if R["dnt"] == nt)
                         for nt in NTYPES}
            for tg in REL_ORDER:
                R = rel_by_tag[tg]
                dnt = R["dnt"]
                if dnt not in cleared:
                    cs, sh = nt_off[dnt], shard[dnt]
                    nc.vector.memset(agg[:od, cs:cs + sh], 0.0)
                    cleared.add(dnt)
                emit_relation(l, R)
                remaining[dnt] -= 1
                if remaining[dnt] == 0:
                    emit_finish_ntype(l, dnt)

        emit_embedding()
        emit_layer(0)
        emit_layer(1)
        emit_layer(2)

    nc.compile()
    return nc


# ---------------------------------------------------------------------------
# entry point
# ---------------------------------------------------------------------------

def _install_ntff_hook():
    if "antenv.axon_hooks" in sys.modules:
        return
    mod = types.ModuleType("antenv.axon_hooks")
    mod._hook = None
    mod.set_axon_ntff_profile_hook = lambda h: setattr(mod, "_hook", h)
    mod.get_axon_ntff_profile_hook = lambda: mod._hook
    sys.modules["antenv.axon_hooks"] = mod
    try:
        import antenv
        antenv.axon_hooks = mod
        from trn_agent_boot.trn_boot import _ntff_profile_via_ctypes
        hook = _ntff_profile_via_ctypes("/opt/axon/libaxon_pjrt.so")
        if hook is not None:
            mod.set_axon_ntff_profile_hook(hook)
    except Exception:
        pass


def run(inputs, cfg=CFG, trace=False, tmpdir=None):
    S, percore = preprocess(cfg, inputs)
    nc = build(S)
    _install_ntff_hook()
    from concourse import bass_utils
    bass_utils.upload_artifacts = lambda d: d
    res = bass_utils.run_bass_kernel_spmd(
        nc, percore, list(range(cfg["NCORE"])), trace=trace, tmpdir=tmpdir,
        trace_cores=[0] if trace else None)
    ncore = cfg["NCORE"]
    shard = {nt: cfg["N"][nt] // ncore for nt in NTYPES}
    outs = []
    o = 0
    for nt in NTYPES:
        parts = [res.results[c]["out"][o:o + shard[nt]] for c in range(ncore)]
        outs.append(np.concatenate(parts, 0))
        o += shard[nt]
    full = np.concatenate(outs, 0).astype(np.float32)
    run.last_exec_time_ns = res.exec_time_ns
    return full


def kernel(**inputs):
    return run(inputs)


# revision 18
# speedup vs baseline: 1.0203x; 1.0132x over previous
"""Trainium2 Bass kernel for nn_BaseRGCNHetero (3-layer heterogeneous RGCN).

Strategy (8 NeuronCores, SPMD):
  - Destination-shard the nodes: core c owns rows [c*N/8, (c+1)*N/8) of every
    node type; all edges whose dst is in the shard are processed there, so
    per-relation aggregates need no cross-core reduction.
  - Aggregate-first algebra: agg[dst] = (sum_{e->dst} h[src]) @ W_r * inv_deg,
    sharing one bf16 gather table per source ntype (drug, gene) per layer.
  - After each layer the drug/gene h-shards are AllGathered (bf16) into
    per-core DRAM gather tables for the next layer.  Relations are ordered
    ddr -> dg -> gg -> dd -> gd so each ntype's activation + AllGather fires
    as early as possible and overlaps the remaining relations' gathers.
  - Segment sums: host lays edges out as a padded, degree-bucketed slot
    stream per (relation, index bank, 128-dst window, 8-dst subgroup).
    dma_gather (bf16, transpose=True) produces feature-major tiles; VectorE
    tensor_reduce over the innermost (slot) axis yields segment sums.  Pad
    slots point at an all-zero table row.  Tables over 32768 rows split into
    two int16 index banks; each bank gets its own per-window degree sort so
    subgroup depth padding stays low.
  - Per (relation, bank, window): a one-hot "unpermute * inv_deg" matrix is
    built by a fused tensor_scalar(is_equal, mult); two matmuls apply W_r and
    the window permutation back to natural dst order, accumulating into a
    feature-major fp32 SBUF accumulator (bank contributions sum there); the
    self-loop h @ L is one more matmul; bias+relu is a fused ScalarE
    activation per window.
"""
import sys
import types
import numpy as np
import ml_dtypes
from contextlib import ExitStack

import concourse.bass as bass
import concourse.bacc as bacc
import concourse.tile as tile
from concourse import mybir, library_config

BF16 = ml_dtypes.bfloat16
P = 128
SUBG = 16          # dsts per reduce subgroup
NSUB = P // SUBG   # subgroups per window
GCAP = 5120        # target max slots per dma_gather

CFG = dict(
    N={"drug": 20000, "gene": 50000, "disease": 10000},
    MOD={"drug": 1024, "gene": 768, "disease": 512},
    D_IN=128, D_H=128, D_OUT=64,
    RELS=[("drug", "disease", "dd"), ("drug", "drug", "ddr"),
          ("drug", "gene", "dg"), ("gene", "disease", "gd"),
          ("gene", "gene", "gg")],
    NCORE=8,
)

NTYPES = ("drug", "gene", "disease")
SRC_NTYPES = ("drug", "gene")
# processing order inside a layer: drug-dst first (unlocks drug AllGather),
# then gene-dst, then disease-dst (no AllGather needed).
REL_ORDER = ["ddr", "dg", "gg", "dd", "gd"]


# ---------------------------------------------------------------------------
# host-side preprocessing
# ---------------------------------------------------------------------------

def _pack_idx(stream):
    """int array (len % 128 == 0) -> dma_gather idx layout [128, len/16] int16:
    idx i at (i%16, i//16), replicated across the 8 groups of 16 partitions."""
    n = stream.size
    v = stream.astype(np.int16).reshape(n // 16, 16).T
    return np.tile(v, (8, 1))


def preprocess(cfg, inputs):
    ncore = cfg["NCORE"]
    shard = {nt: cfg["N"][nt] // ncore for nt in NTYPES}
    nw = {nt: -(-shard[nt] // P) for nt in NTYPES}

    S = dict(cfg=cfg, nw=nw, shard=shard, rels=[])
    percore = [dict() for _ in range(ncore)]

    BANK = 32768
    for r, (snt, dnt, tag) in enumerate(cfg["RELS"]):
        src = np.asarray(inputs["e_" + tag + "_s"]).astype(np.int64)
        dst = np.asarray(inputs["e_" + tag + "_d"]).astype(np.int64)
        trows = cfg["N"][snt] + 2
        # table row 0 is all-zero (pad target); banks split the int16 idx range
        if trows <= BANK:
            banks = [(0, trows)]
        else:
            banks = [(0, BANK), (BANK - 1, trows)]  # overlap row BANK-1 unused;
            # bank1 pad slots use relative row 0 -> absolute BANK-1 (a real
            # node!), so bank1 pads instead point at relative (trows-1-b0)
            # which is the trailing all-zero row.
        nbank = len(banks)
        NW = nw[dnt]
        dsh = shard[dnt]
        npad = NW * P

        core_of = dst // dsh
        deg_all = np.bincount(dst, minlength=cfg["N"][dnt]).astype(np.int64)
        row_all = src + 1
        bank_of = np.zeros(src.size, np.int64)
        for b, (b0, b1) in enumerate(banks):
            bank_of[(row_all >= b0 + (1 if b else 0)) & (row_all < b1)] = b
        pad_rel = [(trows - 1) - b0 for b0, b1 in banks]  # trailing zero row
        pad_rel[0] = 0                                    # leading zero row

        # per-bank, per-core window-local orderings by per-bank degree
        orders = np.zeros((nbank, ncore, npad), np.int64)
        perm_cols = np.zeros((nbank, ncore, NW, P), np.int32)
        invdeg_cols = np.zeros((nbank, ncore, NW, P), np.float32)
        dcnt = np.zeros((nbank, ncore, NW, P), np.int64)
        dn = np.arange(npad)
        for c in range(ncore):
            m = core_of == c
            ld_all = dst[m] - c * dsh
            deg_pad = np.zeros(npad, np.int64)
            deg_pad[:dsh] = deg_all[c * dsh:(c + 1) * dsh]
            ivfull = (1.0 / np.maximum(deg_pad, 1.0)).astype(np.float32)
            ivfull[dsh:] = 0.0
            for b in range(nbank):
                cb = np.bincount(ld_all[bank_of[m] == b], minlength=npad)
                order = np.lexsort((dn, -cb, dn // P))
                orders[b, c] = order
                perm_cols[b, c] = (order % P).reshape(NW, P)
                iv = ivfull[order].copy()
                iv[cb[order] == 0] = 0.0   # no bank-b edges -> contribute 0
                invdeg_cols[b, c] = iv.reshape(NW, P)
                dcnt[b, c] = cb[order].reshape(NW, P)

        # subgroup depths, common across cores; window block sizes % 128
        dq = np.zeros((NW, NSUB, nbank), np.int64)
        for q in range(NSUB):
            dq[:, q, :] = dcnt[:, :, :, q * SUBG:(q + 1) * SUBG].max(
                axis=(1, 3)).T

        # block layout: bank-major, then window, subgroup; greedy gathers.
        # Each gather is padded to a multiple of 128 slots at close time
        # (num_idxs % 128 requirement); pad slots belong to no block so
        # they are gathered but never reduced.
        blocks = []
        OFF = np.full((NW, NSUB, nbank), -1, np.int64)
        off = 0
        gathers = []

        def close_gather(b, gstart, off):
            off += (-(off - gstart)) % P
            gathers.append((b, gstart, off - gstart))
            return off

        for b in range(nbank):
            gstart, gslots = off, 0
            for w in range(NW):
                wslots = int(dq[w, :, b].sum()) * SUBG
                if wslots == 0:
                    continue
                if gslots + wslots > GCAP and gslots > 0:
                    off = close_gather(b, gstart, off)
                    gstart, gslots = off, 0
                for q in range(NSUB):
                    if dq[w, q, b] > 0:
                        blocks.append((w, b, q, int(dq[w, q, b]), off))
                        OFF[w, q, b] = off
                        off += int(dq[w, q, b]) * SUBG
                gslots += wslots
            if gslots > 0:
                off = close_gather(b, gstart, off)
        nslots = max(off, P)
        maxg = max((g[2] for g in gathers), default=P)

        for c in range(ncore):
            stream = np.zeros(nslots, np.int64)
            for (w, b, q, d, o) in blocks:
                stream[o:o + d * SUBG] = pad_rel[b]
            m = core_of == c
            sm_row = row_all[m]
            sm_bank = bank_of[m]
            ld = dst[m] - c * dsh
            for b, (b0, b1) in enumerate(banks):
                inb = sm_bank == b
                if not inb.any():
                    continue
                rel_row = sm_row[inb] - b0
                dp_of = np.zeros(npad, np.int64)
                dp_of[orders[b, c]] = np.arange(npad)
                e_dp = dp_of[ld[inb]]
                e_w, e_dpw = e_dp // P, e_dp % P
                e_q, e_i = e_dpw // SUBG, e_dpw % SUBG
                so = np.argsort(e_dp, kind="stable")
                ks = e_dp[so]
                starts = np.r_[0, np.flatnonzero(np.diff(ks)) + 1]
                sizes = np.diff(np.r_[starts, ks.size])
                cum = np.arange(ks.size) - np.repeat(starts, sizes)
                e_j = np.empty(ks.size, np.int64)
                e_j[so] = cum
                d_arr = dq[e_w, e_q, b]
                pos = OFF[e_w, e_q, b] + e_i * d_arr + e_j
                assert (pos >= 0).all() and (e_j < d_arr).all()
                stream[pos] = rel_row
            percore[c][f"idx_{tag}"] = _pack_idx(stream)
            # perm/invdeg stored [P, nbank*NW], bank-major columns
            percore[c][f"perm_{tag}"] = np.ascontiguousarray(
                perm_cols[:, c].astype(np.float32).reshape(
                    nbank * NW, P).T)
            percore[c][f"invdeg_{tag}"] = np.ascontiguousarray(
                invdeg_cols[:, c].reshape(nbank * NW, P).T)

        S["rels"].append(dict(r=r, snt=snt, dnt=dnt, tag=tag, NW=NW,
                              banks=banks, nbank=nbank, blocks=blocks,
                              gathers=gathers, nslots=nslots, maxg=maxg))

    for nt in NTYPES:
        x = np.asarray(inputs["x_" + nt])
        for c in range(ncore):
            sh = shard[nt]
            percore[c][f"xT_{nt}"] = np.ascontiguousarray(
                x[c * sh:(c + 1) * sh].T).astype(BF16)

    com = dict()
    for nt in NTYPES:
        com[f"We_{nt}"] = np.asarray(inputs["We_" + nt]).astype(BF16)
        com[f"be_{nt}"] = np.asarray(inputs["be_" + nt]).astype(
            np.float32).reshape(-1, 1)
    for l in range(3):
        com[f"W{l}"] = np.asarray(inputs[f"W{l}"]).astype(BF16)
        com[f"L{l}"] = np.asarray(inputs[f"L{l}"]).astype(BF16)
        com[f"b{l}"] = np.asarray(inputs[f"b{l}"]).astype(np.float32).reshape(-1, 1)
    com["iota"] = np.tile(np.arange(P, dtype=np.float32), (P, 1))
    for c in range(ncore):
        percore[c].update(com)
    return S, percore


# ---------------------------------------------------------------------------
# device program
# ---------------------------------------------------------------------------

def build(S):
    cfg = S["cfg"]
    ncore = cfg["NCORE"]
    nw, shard = S["nw"], S["shard"]
    DH, DOUT = cfg["D_H"], cfg["D_OUT"]
    NREL = len(cfg["RELS"])
    nsh_tot = sum(shard.values())
    maxg_all = max(R["maxg"] for R in S["rels"])
    maxw_cols = max(nw[nt] for nt in NTYPES) * P

    nc = bacc.Bacc("TRN2", target_bir_lowering=False, debug=False,
                   num_devices=ncore, dynamic_dma_scratch_size=32768)

    par = {}
    for nt in NTYPES:
        par[f"xT_{nt}"] = nc.declare_dram_parameter(
            f"xT_{nt}", [cfg["MOD"][nt], shard[nt]], mybir.dt.bfloat16, False)
        par[f"We_{nt}"] = nc.declare_dram_parameter(
            f"We_{nt}", [cfg["MOD"][nt], cfg["D_IN"]], mybir.dt.bfloat16, False)
        par[f"be_{nt}"] = nc.declare_dram_parameter(
            f"be_{nt}", [cfg["D_IN"], 1], mybir.dt.float32, False)
    for l in range(3):
        od = DOUT if l == 2 else DH
        par[f"W{l}"] = nc.declare_dram_parameter(
            f"W{l}", [NREL, DH, od], mybir.dt.bfloat16, False)
        par[f"L{l}"] = nc.declare_dram_parameter(
            f"L{l}", [DH, od], mybir.dt.bfloat16, False)
        par[f"b{l}"] = nc.declare_dram_parameter(
            f"b{l}", [od, 1], mybir.dt.float32, False)
    par["iota"] = nc.declare_dram_parameter("iota", [P, P], mybir.dt.float32, False)
    for R in S["rels"]:
        tg = R["tag"]
        par[f"idx_{tg}"] = nc.declare_dram_parameter(
            f"idx_{tg}", [P, R["nslots"] // 16], mybir.dt.int16, False)
        par[f"perm_{tg}"] = nc.declare_dram_parameter(
            f"perm_{tg}", [P, R["nbank"] * R["NW"]], mybir.dt.float32, False)
        par[f"invdeg_{tg}"] = nc.declare_dram_parameter(
            f"invdeg_{tg}", [P, R["nbank"] * R["NW"]], mybir.dt.float32, False)
    out_par = nc.declare_dram_parameter("out", [nsh_tot, DOUT],
                                        mybir.dt.float32, True)

    agin, tabs = {}, {}
    for l in range(3):
        for nt in SRC_NTYPES:
            agin[(l, nt)] = nc.dram_tensor(
                f"agin{l}_{nt}", [shard[nt], DH], mybir.dt.bfloat16)
            tabs[(l, nt)] = nc.dram_tensor(
                f"tab{l}_{nt}", [cfg["N"][nt] + 2, DH], mybir.dt.bfloat16,
                addr_space="Shared")

    rel_by_tag = {R["tag"]: R for R in S["rels"]}

    with ExitStack() as ctx:
        tc = ctx.enter_context(tile.TileContext(nc))
        nc.gpsimd.load_library(library_config.mlp)

        const = ctx.enter_context(tc.tile_pool(name="const", bufs=1))
        persist = ctx.enter_context(tc.tile_pool(name="persist", bufs=1))
        gpool = ctx.enter_context(tc.tile_pool(name="gpool", bufs=3))
        ipool = ctx.enter_context(tc.tile_pool(name="ipool", bufs=4))
        xpool = ctx.enter_context(tc.tile_pool(name="xpool", bufs=2))
        wpool = ctx.enter_context(tc.tile_pool(name="wpool", bufs=4))
        prpool = ctx.enter_context(tc.tile_pool(name="prpool", bufs=2))
        pst = ctx.enter_context(tc.tile_pool(name="pst", bufs=2, space="PSUM"))
        ps1 = ctx.enter_context(tc.tile_pool(name="ps1", bufs=2, space="PSUM"))
        ps2 = ctx.enter_context(tc.tile_pool(name="ps2", bufs=2, space="PSUM"))
        psE = ctx.enter_context(tc.tile_pool(name="psE", bufs=2, space="PSUM"))

        sb_iota = const.tile([P, P], mybir.dt.float32)
        nc.sync.dma_start(sb_iota[:], par["iota"][:])
        identity = const.tile([P, P], mybir.dt.float32)
        from concourse.masks import make_identity
        make_identity(nc, identity[:])
        identity16 = const.tile([P, P], mybir.dt.bfloat16)
        nc.vector.tensor_copy(identity16[:], identity[:])

        sb_W, sb_L, sb_b = {}, {}, {}
        for l in range(3):
            od = DOUT if l == 2 else DH
            t = const.tile([DH, NREL, od], mybir.dt.bfloat16, tag=f"W{l}")
            nc.sync.dma_start(t[:], par[f"W{l}"][:].rearrange("r k o -> k r o"))
            sb_W[l] = t
            sb_L[l] = const.tile([DH, od], mybir.dt.bfloat16, tag=f"L{l}", name=f"L{l}")
            nc.sync.dma_start(sb_L[l][:], par[f"L{l}"][:])
            sb_b[l] = const.tile([od, 1], mybir.dt.float32, tag=f"b{l}", name=f"b{l}")
            nc.sync.dma_start(sb_b[l][:], par[f"b{l}"][:])
        sb_meta = {}
        for R in S["rels"]:
            tg = R["tag"]
            pm = const.tile([P, R["nbank"] * R["NW"]], mybir.dt.float32,
                            tag=f"pm_{tg}")
            nc.sync.dma_start(pm[:], par[f"perm_{tg}"][:])
            iv = const.tile([P, R["nbank"] * R["NW"]], mybir.dt.float32,
                            tag=f"iv_{tg}")
            nc.sync.dma_start(iv[:], par[f"invdeg_{tg}"][:])
            sb_meta[tg] = (pm, iv)

        zrow = const.tile([1, DH], mybir.dt.bfloat16)
        nc.vector.memset(zrow[:], 0.0)
        for l in range(3):
            for nt in SRC_NTYPES:
                n = cfg["N"][nt]
                nc.sync.dma_start(tabs[(l, nt)][0:1, :], zrow[:])
                nc.sync.dma_start(tabs[(l, nt)][n + 1:n + 2, :], zrow[:])

        hT = [persist.tile([DH, nsh_tot], mybir.dt.bfloat16, tag=f"hT{i}",
                           name=f"hT{i}")
              for i in range(2)]
        nt_off, o = {}, 0
        for nt in NTYPES:
            nt_off[nt] = o
            o += shard[nt]
        agg = persist.tile([DH, nsh_tot], mybir.dt.float32, tag="agg")

        def emit_ag(l, nt):
            """Transpose this core's h shard of ntype nt and AllGather it
            into the layer-l gather table."""
            sh = shard[nt]
            for w0 in range(0, sh, P):
                cols = min(P, sh - w0)
                src = hT[l % 2][:, nt_off[nt] + w0:nt_off[nt] + w0 + cols]
                pt = pst.tile([P, P], mybir.dt.bfloat16, tag="tp", name="pt16")
                nc.tensor.transpose(pt[:cols, :DH], src, identity16[:])
                stg = wpool.tile([P, DH], mybir.dt.bfloat16, tag="agstg")
                nc.vector.tensor_copy(stg[:cols, :], pt[:cols, :DH])
                nc.sync.dma_start(agin[(l, nt)][w0:w0 + cols, :],
                                  stg[:cols, :])
            nc.gpsimd.collective_compute(
                "AllGather", mybir.AluOpType.bypass,
                replica_groups=[list(range(ncore))],
                ins=[agin[(l, nt)][:]],
                outs=[tabs[(l, nt)][1:cfg["N"][nt] + 1]],
            )

        def emit_embedding():
            for nt in ("drug", "gene", "disease"):
                mod, sh = cfg["MOD"][nt], shard[nt]
                kt = mod // P
                sb_we = xpool.tile([P, 8, cfg["D_IN"]], mybir.dt.bfloat16, tag="we")
                nc.sync.dma_start(
                    sb_we[:, :kt, :],
                    par[f"We_{nt}"][:].rearrange("(k p) f -> p k f", p=P))
                sb_be = wpool.tile([cfg["D_IN"], 1], mybir.dt.float32, tag="be")
                nc.sync.dma_start(sb_be[:], par[f"be_{nt}"][:])
                for n0 in range(0, sh, 512):
                    n1 = min(n0 + 512, sh)
                    cols = n1 - n0
                    xt = xpool.tile([P, 8, 512], mybir.dt.bfloat16, tag="xt")
                    nc.sync.dma_start(
                        xt[:, :kt, :cols],
                        par[f"xT_{nt}"][:].rearrange(
                            "(k p) n -> p k n", p=P)[:, :, n0:n1])
                    pe = psE.tile([P, 512], mybir.dt.float32, tag="emb")
                    for k in range(kt):
                        nc.tensor.matmul(pe[:, :cols], sb_we[:, k, :],
                                         xt[:, k, :cols],
                                         start=(k == 0), stop=(k == kt - 1))
                    nc.scalar.activation(
                        hT[0][:, nt_off[nt] + n0:nt_off[nt] + n1], pe[:, :cols],
                        mybir.ActivationFunctionType.Identity, bias=sb_be[:])
                if nt in SRC_NTYPES:
                    emit_ag(0, nt)

        def emit_relation(l, R):
            """Per bank: gathers + segment sums into praw, then per-window
            W_r + unpermute*invdeg matmuls accumulated into agg columns of
            R's dst ntype (bank contributions sum in agg)."""
            od = DOUT if l == 2 else DH
            tg, snt, dnt, r, NW = R["tag"], R["snt"], R["dnt"], R["r"], R["NW"]
            tab = tabs[(l, snt)]
            pm, iv = sb_meta[tg]
            blk_by_g = {gi: [] for gi in range(len(R["gathers"]))}
            for blk in R["blocks"]:
                for gi, (gb, goff, gslots) in enumerate(R["gathers"]):
                    if gb == blk[1] and goff <= blk[4] < goff + gslots:
                        blk_by_g[gi].append(blk)
                        break
            for b, (b0, b1) in enumerate(R["banks"]):
                praw = prpool.tile([P, maxw_cols], mybir.dt.bfloat16,
                                   tag="praw")
                written = np.zeros((NW, NSUB), bool)
                for gi, (gb, goff, gslots) in enumerate(R["gathers"]):
                    if gb != b:
                        continue
                    sbi = ipool.tile([P, maxg_all // 16], mybir.dt.int16,
                                     tag="idx")
                    nc.sync.dma_start(
                        sbi[:, :gslots // 16],
                        par[f"idx_{tg}"][:, goff // 16:(goff + gslots) // 16])
                    gt = gpool.tile([P, 1, maxg_all], mybir.dt.bfloat16,
                                    tag="gat")
                    nc.gpsimd.dma_gather(
                        out_ap=gt[:, :, :gslots], in_ap=tab[b0:b1],
                        idxs_ap=sbi[:, :gslots // 16],
                        num_idxs=gslots, num_idxs_reg=gslots,
                        elem_size=DH, transpose=True,
                        single_packet=(gslots <= 992))
                    for (w, bb, q, d, off) in blk_by_g[gi]:
                        loc = off - goff
                        view = gt[:, 0, loc:loc + d * SUBG].rearrange(
                            "p (n d) -> p n d", d=d)
                        cols = slice(w * P + q * SUBG, w * P + (q + 1) * SUBG)
                        with nc.allow_low_precision(
                                reason="DVE reduces in fp32; bf16 rounding "
                                       "applies once at output"):
                            nc.vector.tensor_reduce(
                                praw[:, cols], view, axis=mybir.AxisListType.X,
                                op=mybir.AluOpType.add)
                        written[w, q] = True
                for w in range(NW):
                    for q in range(NSUB):
                        if not written[w, q]:
                            nc.vector.memset(
                                praw[:, w * P + q * SUBG:
                                     w * P + (q + 1) * SUBG], 0.0)
                for w in range(NW):
                    wb = b * NW + w
                    Sp = wpool.tile([P, P], mybir.dt.bfloat16, tag="Sperm")
                    nc.vector.tensor_scalar(
                        Sp[:], sb_iota[:], pm[:, wb:wb + 1], iv[:, wb:wb + 1],
                        op0=mybir.AluOpType.is_equal, op1=mybir.AluOpType.mult)
                    p1 = ps1.tile([P, DH], mybir.dt.float32, tag="out1")
                    nc.tensor.matmul(p1[:, :od], praw[:, w * P:(w + 1) * P],
                                     sb_W[l][:, r, :],
                                     start=True, stop=True)
                    o1 = wpool.tile([P, DH], mybir.dt.bfloat16, tag="o1")
                    nc.vector.tensor_copy(o1[:, :od], p1[:, :od])
                    p2 = ps2.tile([P, P], mybir.dt.float32, tag="out2")
                    nc.tensor.matmul(p2[:od, :], o1[:, :od], Sp[:],
                                     start=True, stop=True)
                    cs = nt_off[dnt] + w * P
                    ce = min(cs + P, nt_off[dnt] + shard[dnt])
                    nc.vector.tensor_add(agg[:od, cs:ce], agg[:od, cs:ce],
                                         p2[:od, :ce - cs])

        def emit_finish_ntype(l, nt):
            """Self-loop + bias (+relu) for ntype nt; write hT (or output)."""
            od = DOUT if l == 2 else DH
            sh = shard[nt]
            for w0 in range(0, sh, P):
                cols = min(P, sh - w0)
                cs = nt_off[nt] + w0
                p2 = ps2.tile([P, P], mybir.dt.float32, tag="out2")
                nc.tensor.matmul(p2[:od, :cols], sb_L[l][:],
                                 hT[l % 2][:, cs:cs + cols],
                                 start=True, stop=True)
                nc.vector.tensor_add(agg[:od, cs:cs + cols],
                                     agg[:od, cs:cs + cols],
                                     p2[:od, :cols])
                if l < 2:
                    nc.scalar.activation(
                        hT[(l + 1) % 2][:od, cs:cs + cols],
                        agg[:od, cs:cs + cols],
                        mybir.ActivationFunctionType.Relu, bias=sb_b[l][:])
                else:
                    fin = wpool.tile([P, P], mybir.dt.float32, tag="fin")
                    nc.scalar.activation(
                        fin[:od, :cols], agg[:od, cs:cs + cols],
                        mybir.ActivationFunctionType.Identity,
                        bias=sb_b[l][:])
                    pt = pst.tile([P, P], mybir.dt.float32, tag="tp")
                    nc.tensor.transpose(pt[:cols, :od], fin[:od, :cols],
                                        identity[:od, :od])
                    stg = wpool.tile([P, DOUT], mybir.dt.float32, tag="ostg")
                    nc.vector.tensor_copy(stg[:cols, :], pt[:cols, :od])
                    nc.sync.dma_start(out_par[cs:cs + cols, :],
                                      stg[:cols, :])
            if l < 2 and nt in SRC_NTYPES:
                emit_ag(l + 1, nt)

        def emit_layer(l):
            od = DOUT if l == 2 else DH
            # first relation targeting each dst ntype clears its agg columns
            cleared = set()
            # ntype completed when all relations targeting it are done
            remaining = {nt: sum(1 for R in S["rels"] if R["dnt"] == nt)
                         for nt in NTYPES}
            for tg in REL_ORDER:
                R = rel_by_tag[tg]
                dnt = R["dnt"]
                if dnt not in cleared:
                    cs, sh = nt_off[dnt], shard[dnt]
                    nc.vector.memset(agg[:od, cs:cs + sh], 0.0)
                    cleared.add(dnt)
                emit_relation(l, R)
                remaining[dnt] -= 1
                if remaining[dnt] == 0:
                    emit_finish_ntype(l, dnt)

        emit_embedding()
        emit_layer(0)
        emit_layer(1)
        emit_layer(2)

    nc.compile()
    return nc


# ---------------------------------------------------------------------------
# entry point
# ---------------------------------------------------------------------------

def _install_ntff_hook():
    if "antenv.axon_hooks" in sys.modules:
        return
    mod = types.ModuleType("antenv.axon_hooks")
    mod._hook = None
    mod.set_axon_ntff_profile_hook = lambda h: setattr(mod, "_hook", h)
    mod.get_axon_ntff_profile_hook = lambda: mod._hook
    sys.modules["antenv.axon_hooks"] = mod
    try:
        import antenv
        antenv.axon_hooks = mod
        from trn_agent_boot.trn_boot import _ntff_profile_via_ctypes
        hook = _ntff_profile_via_ctypes("/opt/axon/libaxon_pjrt.so")
        if hook is not None:
            mod.set_axon_ntff_profile_hook(hook)
    except Exception:
        pass


def run(inputs, cfg=CFG, trace=False, tmpdir=None):
    S, percore = preprocess(cfg, inputs)
    nc = build(S)
    _install_ntff_hook()
    from concourse import bass_utils
    bass_utils.upload_artifacts = lambda d: d
    res = bass_utils.run_bass_kernel_spmd(
        nc, percore, list(range(cfg["NCORE"])), trace=trace, tmpdir=tmpdir,
        trace_cores=[0] if trace else None)
    ncore = cfg["NCORE"]
    shard = {nt: cfg["N"][nt] // ncore for nt in NTYPES}
    outs = []
    o = 0
    for nt in NTYPES:
        parts = [res.results[c]["out"][o:o + shard[nt]] for c in range(ncore)]
        outs.append(np.concatenate(parts, 0))
        o += shard[nt]
    full = np.concatenate(outs, 0).astype(np.float32)
    run.last_exec_time_ns = res.exec_time_ns
    return full


def kernel(**inputs):
    return run(inputs)


# revision 20
# speedup vs baseline: 1.0255x; 1.0051x over previous
"""Trainium2 Bass kernel for nn_BaseRGCNHetero (3-layer heterogeneous RGCN).

Strategy (8 NeuronCores, SPMD):
  - Destination-shard the nodes: core c owns rows [c*N/8, (c+1)*N/8) of every
    node type; all edges whose dst is in the shard are processed there, so
    per-relation aggregates need no cross-core reduction.
  - Aggregate-first algebra: agg[dst] = (sum_{e->dst} h[src]) @ W_r * inv_deg,
    sharing one bf16 gather table per source ntype (drug, gene) per layer.
  - After each layer the drug/gene h-shards are AllGathered (bf16) into
    per-core DRAM gather tables for the next layer.  Relations are ordered
    ddr -> dg -> gg -> dd -> gd so each ntype's activation + AllGather fires
    as early as possible and overlaps the remaining relations' gathers.
  - Segment sums: host lays edges out as a padded, degree-bucketed slot
    stream per (relation, index bank, 128-dst window, 8-dst subgroup).
    dma_gather (bf16, transpose=True) produces feature-major tiles; VectorE
    tensor_reduce over the innermost (slot) axis yields segment sums.  Pad
    slots point at an all-zero table row.  Tables over 32768 rows split into
    two int16 index banks; each bank gets its own per-window degree sort so
    subgroup depth padding stays low.
  - Per (relation, bank, window): a one-hot "unpermute * inv_deg" matrix is
    built by a fused tensor_scalar(is_equal, mult); two matmuls apply W_r and
    the window permutation back to natural dst order, accumulating into a
    feature-major fp32 SBUF accumulator (bank contributions sum there); the
    self-loop h @ L is one more matmul; bias+relu is a fused ScalarE
    activation per window.
"""
import sys
import types
import numpy as np
import ml_dtypes
from contextlib import ExitStack

import concourse.bass as bass
import concourse.bacc as bacc
import concourse.tile as tile
from concourse import mybir, library_config

BF16 = ml_dtypes.bfloat16
P = 128
SUBG = 16          # dsts per reduce subgroup
NSUB = P // SUBG   # subgroups per window
GCAP = 5120        # target max slots per dma_gather

CFG = dict(
    N={"drug": 20000, "gene": 50000, "disease": 10000},
    MOD={"drug": 1024, "gene": 768, "disease": 512},
    D_IN=128, D_H=128, D_OUT=64,
    RELS=[("drug", "disease", "dd"), ("drug", "drug", "ddr"),
          ("drug", "gene", "dg"), ("gene", "disease", "gd"),
          ("gene", "gene", "gg")],
    NCORE=8,
)

NTYPES = ("drug", "gene", "disease")
SRC_NTYPES = ("drug", "gene")
# processing order inside a layer: drug-dst first (unlocks drug AllGather),
# then gene-dst, then disease-dst (no AllGather needed).
REL_ORDER = ["ddr", "dg", "gg", "dd", "gd"]


# ---------------------------------------------------------------------------
# host-side preprocessing
# ---------------------------------------------------------------------------

def _pack_idx(stream):
    """int array (len % 128 == 0) -> dma_gather idx layout [128, len/16] int16:
    idx i at (i%16, i//16), replicated across the 8 groups of 16 partitions."""
    n = stream.size
    v = stream.astype(np.int16).reshape(n // 16, 16).T
    return np.tile(v, (8, 1))


def preprocess(cfg, inputs):
    ncore = cfg["NCORE"]
    shard = {nt: cfg["N"][nt] // ncore for nt in NTYPES}
    nw = {nt: -(-shard[nt] // P) for nt in NTYPES}

    S = dict(cfg=cfg, nw=nw, shard=shard, rels=[])
    percore = [dict() for _ in range(ncore)]

    BANK = 32768
    for r, (snt, dnt, tag) in enumerate(cfg["RELS"]):
        src = np.asarray(inputs["e_" + tag + "_s"]).astype(np.int64)
        dst = np.asarray(inputs["e_" + tag + "_d"]).astype(np.int64)
        trows = cfg["N"][snt] + 2
        # table row 0 is all-zero (pad target); banks split the int16 idx range
        if trows <= BANK:
            banks = [(0, trows)]
        else:
            banks = [(0, BANK), (BANK - 1, trows)]  # overlap row BANK-1 unused;
            # bank1 pad slots use relative row 0 -> absolute BANK-1 (a real
            # node!), so bank1 pads instead point at relative (trows-1-b0)
            # which is the trailing all-zero row.
        nbank = len(banks)
        NW = nw[dnt]
        dsh = shard[dnt]
        npad = NW * P

        core_of = dst // dsh
        deg_all = np.bincount(dst, minlength=cfg["N"][dnt]).astype(np.int64)
        row_all = src + 1
        bank_of = np.zeros(src.size, np.int64)
        for b, (b0, b1) in enumerate(banks):
            bank_of[(row_all >= b0 + (1 if b else 0)) & (row_all < b1)] = b
        pad_rel = [(trows - 1) - b0 for b0, b1 in banks]  # trailing zero row
        pad_rel[0] = 0                                    # leading zero row

        # per-bank, per-core window-local orderings by per-bank degree
        orders = np.zeros((nbank, ncore, npad), np.int64)
        perm_cols = np.zeros((nbank, ncore, NW, P), np.int32)
        invdeg_cols = np.zeros((nbank, ncore, NW, P), np.float32)
        dcnt = np.zeros((nbank, ncore, NW, P), np.int64)
        dn = np.arange(npad)
        for c in range(ncore):
            m = core_of == c
            ld_all = dst[m] - c * dsh
            deg_pad = np.zeros(npad, np.int64)
            deg_pad[:dsh] = deg_all[c * dsh:(c + 1) * dsh]
            ivfull = (1.0 / np.maximum(deg_pad, 1.0)).astype(np.float32)
            ivfull[dsh:] = 0.0
            for b in range(nbank):
                cb = np.bincount(ld_all[bank_of[m] == b], minlength=npad)
                order = np.lexsort((dn, -cb, dn // P))
                orders[b, c] = order
                perm_cols[b, c] = (order % P).reshape(NW, P)
                iv = ivfull[order].copy()
                iv[cb[order] == 0] = 0.0   # no bank-b edges -> contribute 0
                invdeg_cols[b, c] = iv.reshape(NW, P)
                dcnt[b, c] = cb[order].reshape(NW, P)

        # subgroup depths, common across cores; window block sizes % 128
        dq = np.zeros((NW, NSUB, nbank), np.int64)
        for q in range(NSUB):
            dq[:, q, :] = dcnt[:, :, :, q * SUBG:(q + 1) * SUBG].max(
                axis=(1, 3)).T

        # block layout: bank-major, then window, subgroup; greedy gathers.
        # Each gather is padded to a multiple of 128 slots at close time
        # (num_idxs % 128 requirement); pad slots belong to no block so
        # they are gathered but never reduced.
        blocks = []
        OFF = np.full((NW, NSUB, nbank), -1, np.int64)
        off = 0
        gathers = []

        def close_gather(b, gstart, off):
            off += (-(off - gstart)) % P
            gathers.append((b, gstart, off - gstart))
            return off

        for b in range(nbank):
            gstart, gslots = off, 0
            for w in range(NW):
                wslots = int(dq[w, :, b].sum()) * SUBG
                if wslots == 0:
                    continue
                if gslots + wslots > GCAP and gslots > 0:
                    off = close_gather(b, gstart, off)
                    gstart, gslots = off, 0
                for q in range(NSUB):
                    if dq[w, q, b] > 0:
                        blocks.append((w, b, q, int(dq[w, q, b]), off))
                        OFF[w, q, b] = off
                        off += int(dq[w, q, b]) * SUBG
                gslots += wslots
            if gslots > 0:
                off = close_gather(b, gstart, off)
        nslots = max(off, P)
        maxg = max((g[2] for g in gathers), default=P)

        for c in range(ncore):
            stream = np.zeros(nslots, np.int64)
            for (w, b, q, d, o) in blocks:
                stream[o:o + d * SUBG] = pad_rel[b]
            m = core_of == c
            sm_row = row_all[m]
            sm_bank = bank_of[m]
            ld = dst[m] - c * dsh
            for b, (b0, b1) in enumerate(banks):
                inb = sm_bank == b
                if not inb.any():
                    continue
                rel_row = sm_row[inb] - b0
                dp_of = np.zeros(npad, np.int64)
                dp_of[orders[b, c]] = np.arange(npad)
                e_dp = dp_of[ld[inb]]
                e_w, e_dpw = e_dp // P, e_dp % P
                e_q, e_i = e_dpw // SUBG, e_dpw % SUBG
                so = np.argsort(e_dp, kind="stable")
                ks = e_dp[so]
                starts = np.r_[0, np.flatnonzero(np.diff(ks)) + 1]
                sizes = np.diff(np.r_[starts, ks.size])
                cum = np.arange(ks.size) - np.repeat(starts, sizes)
                e_j = np.empty(ks.size, np.int64)
                e_j[so] = cum
                d_arr = dq[e_w, e_q, b]
                pos = OFF[e_w, e_q, b] + e_i * d_arr + e_j
                assert (pos >= 0).all() and (e_j < d_arr).all()
                stream[pos] = rel_row
            percore[c][f"idx_{tag}"] = _pack_idx(stream)
            # perm/invdeg stored [P, nbank*NW], bank-major columns
            percore[c][f"perm_{tag}"] = np.ascontiguousarray(
                perm_cols[:, c].astype(np.float32).reshape(
                    nbank * NW, P).T)
            percore[c][f"invdeg_{tag}"] = np.ascontiguousarray(
                invdeg_cols[:, c].reshape(nbank * NW, P).T)

        S["rels"].append(dict(r=r, snt=snt, dnt=dnt, tag=tag, NW=NW,
                              banks=banks, nbank=nbank, blocks=blocks,
                              gathers=gathers, nslots=nslots, maxg=maxg))

    for nt in NTYPES:
        x = np.asarray(inputs["x_" + nt])
        for c in range(ncore):
            sh = shard[nt]
            percore[c][f"xT_{nt}"] = np.ascontiguousarray(
                x[c * sh:(c + 1) * sh].T).astype(BF16)

    com = dict()
    for nt in NTYPES:
        com[f"We_{nt}"] = np.asarray(inputs["We_" + nt]).astype(BF16)
        com[f"be_{nt}"] = np.asarray(inputs["be_" + nt]).astype(
            np.float32).reshape(-1, 1)
    for l in range(3):
        com[f"W{l}"] = np.asarray(inputs[f"W{l}"]).astype(BF16)
        com[f"L{l}"] = np.asarray(inputs[f"L{l}"]).astype(BF16)
        com[f"b{l}"] = np.asarray(inputs[f"b{l}"]).astype(np.float32).reshape(-1, 1)
    com["iota"] = np.tile(np.arange(P, dtype=np.float32), (P, 1))
    for c in range(ncore):
        percore[c].update(com)
    return S, percore


# ---------------------------------------------------------------------------
# device program
# ---------------------------------------------------------------------------

def build(S):
    cfg = S["cfg"]
    ncore = cfg["NCORE"]
    nw, shard = S["nw"], S["shard"]
    DH, DOUT = cfg["D_H"], cfg["D_OUT"]
    NREL = len(cfg["RELS"])
    nsh_tot = sum(shard.values())
    maxg_all = max(R["maxg"] for R in S["rels"])
    maxw_cols = max(nw[nt] for nt in NTYPES) * P

    nc = bacc.Bacc("TRN2", target_bir_lowering=False, debug=False,
                   num_devices=ncore, dynamic_dma_scratch_size=24576)

    par = {}
    for nt in NTYPES:
        par[f"xT_{nt}"] = nc.declare_dram_parameter(
            f"xT_{nt}", [cfg["MOD"][nt], shard[nt]], mybir.dt.bfloat16, False)
        par[f"We_{nt}"] = nc.declare_dram_parameter(
            f"We_{nt}", [cfg["MOD"][nt], cfg["D_IN"]], mybir.dt.bfloat16, False)
        par[f"be_{nt}"] = nc.declare_dram_parameter(
            f"be_{nt}", [cfg["D_IN"], 1], mybir.dt.float32, False)
    for l in range(3):
        od = DOUT if l == 2 else DH
        par[f"W{l}"] = nc.declare_dram_parameter(
            f"W{l}", [NREL, DH, od], mybir.dt.bfloat16, False)
        par[f"L{l}"] = nc.declare_dram_parameter(
            f"L{l}", [DH, od], mybir.dt.bfloat16, False)
        par[f"b{l}"] = nc.declare_dram_parameter(
            f"b{l}", [od, 1], mybir.dt.float32, False)
    par["iota"] = nc.declare_dram_parameter("iota", [P, P], mybir.dt.float32, False)
    for R in S["rels"]:
        tg = R["tag"]
        par[f"idx_{tg}"] = nc.declare_dram_parameter(
            f"idx_{tg}", [P, R["nslots"] // 16], mybir.dt.int16, False)
        par[f"perm_{tg}"] = nc.declare_dram_parameter(
            f"perm_{tg}", [P, R["nbank"] * R["NW"]], mybir.dt.float32, False)
        par[f"invdeg_{tg}"] = nc.declare_dram_parameter(
            f"invdeg_{tg}", [P, R["nbank"] * R["NW"]], mybir.dt.float32, False)
    out_par = nc.declare_dram_parameter("out", [nsh_tot, DOUT],
                                        mybir.dt.float32, True)

    agin, tabs = {}, {}
    for l in range(3):
        for nt in SRC_NTYPES:
            agin[(l, nt)] = nc.dram_tensor(
                f"agin{l}_{nt}", [shard[nt], DH], mybir.dt.bfloat16)
            tabs[(l, nt)] = nc.dram_tensor(
                f"tab{l}_{nt}", [cfg["N"][nt] + 2, DH], mybir.dt.bfloat16,
                addr_space="Shared")

    rel_by_tag = {R["tag"]: R for R in S["rels"]}

    with ExitStack() as ctx:
        tc = ctx.enter_context(tile.TileContext(nc))
        nc.gpsimd.load_library(library_config.mlp)

        const = ctx.enter_context(tc.tile_pool(name="const", bufs=1))
        persist = ctx.enter_context(tc.tile_pool(name="persist", bufs=1))
        gpool = ctx.enter_context(tc.tile_pool(name="gpool", bufs=4))
        ipool = ctx.enter_context(tc.tile_pool(name="ipool", bufs=4))
        xpool = ctx.enter_context(tc.tile_pool(name="xpool", bufs=2))
        wpool = ctx.enter_context(tc.tile_pool(name="wpool", bufs=4))
        prpool = ctx.enter_context(tc.tile_pool(name="prpool", bufs=2))
        pst = ctx.enter_context(tc.tile_pool(name="pst", bufs=2, space="PSUM"))
        ps1 = ctx.enter_context(tc.tile_pool(name="ps1", bufs=2, space="PSUM"))
        ps2 = ctx.enter_context(tc.tile_pool(name="ps2", bufs=2, space="PSUM"))
        psE = ctx.enter_context(tc.tile_pool(name="psE", bufs=2, space="PSUM"))

        sb_iota = const.tile([P, P], mybir.dt.float32)
        nc.sync.dma_start(sb_iota[:], par["iota"][:])
        identity = const.tile([P, P], mybir.dt.float32)
        from concourse.masks import make_identity
        make_identity(nc, identity[:])
        identity16 = const.tile([P, P], mybir.dt.bfloat16)
        nc.vector.tensor_copy(identity16[:], identity[:])

        sb_W, sb_L, sb_b = {}, {}, {}
        for l in range(3):
            od = DOUT if l == 2 else DH
            t = const.tile([DH, NREL, od], mybir.dt.bfloat16, tag=f"W{l}")
            nc.sync.dma_start(t[:], par[f"W{l}"][:].rearrange("r k o -> k r o"))
            sb_W[l] = t
            sb_L[l] = const.tile([DH, od], mybir.dt.bfloat16, tag=f"L{l}", name=f"L{l}")
            nc.sync.dma_start(sb_L[l][:], par[f"L{l}"][:])
            sb_b[l] = const.tile([od, 1], mybir.dt.float32, tag=f"b{l}", name=f"b{l}")
            nc.sync.dma_start(sb_b[l][:], par[f"b{l}"][:])
        sb_meta = {}
        for R in S["rels"]:
            tg = R["tag"]
            pm = const.tile([P, R["nbank"] * R["NW"]], mybir.dt.float32,
                            tag=f"pm_{tg}")
            nc.sync.dma_start(pm[:], par[f"perm_{tg}"][:])
            iv = const.tile([P, R["nbank"] * R["NW"]], mybir.dt.float32,
                            tag=f"iv_{tg}")
            nc.sync.dma_start(iv[:], par[f"invdeg_{tg}"][:])
            sb_meta[tg] = (pm, iv)

        zrow = const.tile([1, DH], mybir.dt.bfloat16)
        nc.vector.memset(zrow[:], 0.0)
        for l in range(3):
            for nt in SRC_NTYPES:
                n = cfg["N"][nt]
                nc.sync.dma_start(tabs[(l, nt)][0:1, :], zrow[:])
                nc.sync.dma_start(tabs[(l, nt)][n + 1:n + 2, :], zrow[:])

        hT = [persist.tile([DH, nsh_tot], mybir.dt.bfloat16, tag=f"hT{i}",
                           name=f"hT{i}")
              for i in range(2)]
        nt_off, o = {}, 0
        for nt in NTYPES:
            nt_off[nt] = o
            o += shard[nt]
        agg = persist.tile([DH, nsh_tot], mybir.dt.float32, tag="agg")

        def emit_ag(l, nt):
            """Transpose this core's h shard of ntype nt and AllGather it
            into the layer-l gather table."""
            sh = shard[nt]
            for w0 in range(0, sh, P):
                cols = min(P, sh - w0)
                src = hT[l % 2][:, nt_off[nt] + w0:nt_off[nt] + w0 + cols]
                pt = pst.tile([P, P], mybir.dt.bfloat16, tag="tp", name="pt16")
                nc.tensor.transpose(pt[:cols, :DH], src, identity16[:])
                stg = wpool.tile([P, DH], mybir.dt.bfloat16, tag="agstg")
                nc.vector.tensor_copy(stg[:cols, :], pt[:cols, :DH])
                nc.sync.dma_start(agin[(l, nt)][w0:w0 + cols, :],
                                  stg[:cols, :])
            nc.gpsimd.collective_compute(
                "AllGather", mybir.AluOpType.bypass,
                replica_groups=[list(range(ncore))],
                ins=[agin[(l, nt)][:]],
                outs=[tabs[(l, nt)][1:cfg["N"][nt] + 1]],
            )

        def emit_embedding():
            for nt in ("drug", "gene", "disease"):
                mod, sh = cfg["MOD"][nt], shard[nt]
                kt = mod // P
                sb_we = xpool.tile([P, 8, cfg["D_IN"]], mybir.dt.bfloat16, tag="we")
                nc.sync.dma_start(
                    sb_we[:, :kt, :],
                    par[f"We_{nt}"][:].rearrange("(k p) f -> p k f", p=P))
                sb_be = wpool.tile([cfg["D_IN"], 1], mybir.dt.float32, tag="be")
                nc.sync.dma_start(sb_be[:], par[f"be_{nt}"][:])
                for n0 in range(0, sh, 512):
                    n1 = min(n0 + 512, sh)
                    cols = n1 - n0
                    xt = xpool.tile([P, 8, 512], mybir.dt.bfloat16, tag="xt")
                    nc.sync.dma_start(
                        xt[:, :kt, :cols],
                        par[f"xT_{nt}"][:].rearrange(
                            "(k p) n -> p k n", p=P)[:, :, n0:n1])
                    pe = psE.tile([P, 512], mybir.dt.float32, tag="emb")
                    for k in range(kt):
                        nc.tensor.matmul(pe[:, :cols], sb_we[:, k, :],
                                         xt[:, k, :cols],
                                         start=(k == 0), stop=(k == kt - 1))
                    nc.scalar.activation(
                        hT[0][:, nt_off[nt] + n0:nt_off[nt] + n1], pe[:, :cols],
                        mybir.ActivationFunctionType.Identity, bias=sb_be[:])
                if nt in SRC_NTYPES:
                    emit_ag(0, nt)

        def emit_relation(l, R):
            """Per bank: gathers + segment sums into praw, then per-window
            W_r + unpermute*invdeg matmuls accumulated into agg columns of
            R's dst ntype (bank contributions sum in agg)."""
            od = DOUT if l == 2 else DH
            tg, snt, dnt, r, NW = R["tag"], R["snt"], R["dnt"], R["r"], R["NW"]
            tab = tabs[(l, snt)]
            pm, iv = sb_meta[tg]
            blk_by_g = {gi: [] for gi in range(len(R["gathers"]))}
            for blk in R["blocks"]:
                for gi, (gb, goff, gslots) in enumerate(R["gathers"]):
                    if gb == blk[1] and goff <= blk[4] < goff + gslots:
                        blk_by_g[gi].append(blk)
                        break
            for b, (b0, b1) in enumerate(R["banks"]):
                praw = prpool.tile([P, maxw_cols], mybir.dt.bfloat16,
                                   tag="praw")
                written = np.zeros((NW, NSUB), bool)
                for gi, (gb, goff, gslots) in enumerate(R["gathers"]):
                    if gb != b:
                        continue
                    sbi = ipool.tile([P, maxg_all // 16], mybir.dt.int16,
                                     tag="idx")
                    nc.sync.dma_start(
                        sbi[:, :gslots // 16],
                        par[f"idx_{tg}"][:, goff // 16:(goff + gslots) // 16])
                    gt = gpool.tile([P, 1, maxg_all], mybir.dt.bfloat16,
                                    tag="gat")
                    nc.gpsimd.dma_gather(
                        out_ap=gt[:, :, :gslots], in_ap=tab[b0:b1],
                        idxs_ap=sbi[:, :gslots // 16],
                        num_idxs=gslots, num_idxs_reg=gslots,
                        elem_size=DH, transpose=True,
                        single_packet=(gslots <= 992))
                    for (w, bb, q, d, off) in blk_by_g[gi]:
                        loc = off - goff
                        view = gt[:, 0, loc:loc + d * SUBG].rearrange(
                            "p (n d) -> p n d", d=d)
                        cols = slice(w * P + q * SUBG, w * P + (q + 1) * SUBG)
                        with nc.allow_low_precision(
                                reason="DVE reduces in fp32; bf16 rounding "
                                       "applies once at output"):
                            nc.vector.tensor_reduce(
                                praw[:, cols], view, axis=mybir.AxisListType.X,
                                op=mybir.AluOpType.add)
                        written[w, q] = True
                for w in range(NW):
                    for q in range(NSUB):
                        if not written[w, q]:
                            nc.vector.memset(
                                praw[:, w * P + q * SUBG:
                                     w * P + (q + 1) * SUBG], 0.0)
                for w in range(NW):
                    wb = b * NW + w
                    Sp = wpool.tile([P, P], mybir.dt.bfloat16, tag="Sperm")
                    nc.vector.tensor_scalar(
                        Sp[:], sb_iota[:], pm[:, wb:wb + 1], iv[:, wb:wb + 1],
                        op0=mybir.AluOpType.is_equal, op1=mybir.AluOpType.mult)
                    p1 = ps1.tile([P, DH], mybir.dt.float32, tag="out1")
                    nc.tensor.matmul(p1[:, :od], praw[:, w * P:(w + 1) * P],
                                     sb_W[l][:, r, :],
                                     start=True, stop=True)
                    o1 = wpool.tile([P, DH], mybir.dt.bfloat16, tag="o1")
                    nc.vector.tensor_copy(o1[:, :od], p1[:, :od])
                    p2 = ps2.tile([P, P], mybir.dt.float32, tag="out2")
                    nc.tensor.matmul(p2[:od, :], o1[:, :od], Sp[:],
                                     start=True, stop=True)
                    cs = nt_off[dnt] + w * P
                    ce = min(cs + P, nt_off[dnt] + shard[dnt])
                    nc.vector.tensor_add(agg[:od, cs:ce], agg[:od, cs:ce],
                                         p2[:od, :ce - cs])

        def emit_finish_ntype(l, nt):
            """Self-loop + bias (+relu) for ntype nt; write hT (or output)."""
            od = DOUT if l == 2 else DH
            sh = shard[nt]
            for w0 in range(0, sh, P):
                cols = min(P, sh - w0)
                cs = nt_off[nt] + w0
                p2 = ps2.tile([P, P], mybir.dt.float32, tag="out2")
                nc.tensor.matmul(p2[:od, :cols], sb_L[l][:],
                                 hT[l % 2][:, cs:cs + cols],
                                 start=True, stop=True)
                nc.vector.tensor_add(agg[:od, cs:cs + cols],
                                     agg[:od, cs:cs + cols],
                                     p2[:od, :cols])
                if l < 2:
                    nc.scalar.activation(
                        hT[(l + 1) % 2][:od, cs:cs + cols],
                        agg[:od, cs:cs + cols],
                        mybir.ActivationFunctionType.Relu, bias=sb_b[l][:])
                else:
                    fin = wpool.tile([P, P], mybir.dt.float32, tag="fin")
                    nc.scalar.activation(
                        fin[:od, :cols], agg[:od, cs:cs + cols],
                        mybir.ActivationFunctionType.Identity,
                        bias=sb_b[l][:])
                    pt = pst.tile([P, P], mybir.dt.float32, tag="tp")
                    nc.tensor.transpose(pt[:cols, :od], fin[:od, :cols],
                                        identity[:od, :od])
                    stg = wpool.tile([P, DOUT], mybir.dt.float32, tag="ostg")
                    nc.vector.tensor_copy(stg[:cols, :], pt[:cols, :od])
                    nc.sync.dma_start(out_par[cs:cs + cols, :],
                                      stg[:cols, :])
            if l < 2 and nt in SRC_NTYPES:
                emit_ag(l + 1, nt)

        def emit_layer(l):
            od = DOUT if l == 2 else DH
            # first relation targeting each dst ntype clears its agg columns
            cleared = set()
            # ntype completed when all relations targeting it are done
            remaining = {nt: sum(1 for R in S["rels"] if R["dnt"] == nt)
                         for nt in NTYPES}
            for tg in REL_ORDER:
                R = rel_by_tag[tg]
                dnt = R["dnt"]
                if dnt not in cleared:
                    cs, sh = nt_off[dnt], shard[dnt]
                    nc.vector.memset(agg[:od, cs:cs + sh], 0.0)
                    cleared.add(dnt)
                emit_relation(l, R)
                remaining[dnt] -= 1
                if remaining[dnt] == 0:
                    emit_finish_ntype(l, dnt)

        emit_embedding()
        emit_layer(0)
        emit_layer(1)
        emit_layer(2)

    nc.compile()
    return nc


# ---------------------------------------------------------------------------
# entry point
# ---------------------------------------------------------------------------

def _install_ntff_hook():
    if "antenv.axon_hooks" in sys.modules:
        return
    mod = types.ModuleType("antenv.axon_hooks")
    mod._hook = None
    mod.set_axon_ntff_profile_hook = lambda h: setattr(mod, "_hook", h)
    mod.get_axon_ntff_profile_hook = lambda: mod._hook
    sys.modules["antenv.axon_hooks"] = mod
    try:
        import antenv
        antenv.axon_hooks = mod
        from trn_agent_boot.trn_boot import _ntff_profile_via_ctypes
        hook = _ntff_profile_via_ctypes("/opt/axon/libaxon_pjrt.so")
        if hook is not None:
            mod.set_axon_ntff_profile_hook(hook)
    except Exception:
        pass


def run(inputs, cfg=CFG, trace=False, tmpdir=None):
    S, percore = preprocess(cfg, inputs)
    nc = build(S)
    _install_ntff_hook()
    from concourse import bass_utils
    bass_utils.upload_artifacts = lambda d: d
    res = bass_utils.run_bass_kernel_spmd(
        nc, percore, list(range(cfg["NCORE"])), trace=trace, tmpdir=tmpdir,
        trace_cores=[0] if trace else None)
    ncore = cfg["NCORE"]
    shard = {nt: cfg["N"][nt] // ncore for nt in NTYPES}
    outs = []
    o = 0
    for nt in NTYPES:
        parts = [res.results[c]["out"][o:o + shard[nt]] for c in range(ncore)]
        outs.append(np.concatenate(parts, 0))
        o += shard[nt]
    full = np.concatenate(outs, 0).astype(np.float32)
    run.last_exec_time_ns = res.exec_time_ns
    return full


def kernel(**inputs):
    return run(inputs)


# revision 22
# speedup vs baseline: 1.0325x; 1.0069x over previous
"""Trainium2 Bass kernel for nn_BaseRGCNHetero (3-layer heterogeneous RGCN).

Strategy (8 NeuronCores, SPMD):
  - Destination-shard the nodes: core c owns rows [c*N/8, (c+1)*N/8) of every
    node type; all edges whose dst is in the shard are processed there, so
    per-relation aggregates need no cross-core reduction.
  - Aggregate-first algebra: agg[dst] = (sum_{e->dst} h[src]) @ W_r * inv_deg,
    sharing one bf16 gather table per source ntype (drug, gene) per layer.
  - After each layer the drug/gene h-shards are AllGathered (bf16) into
    per-core DRAM gather tables for the next layer.  Relations are ordered
    ddr -> dg -> gg -> dd -> gd so each ntype's activation + AllGather fires
    as early as possible and overlaps the remaining relations' gathers.
  - Segment sums: host lays edges out as a padded, degree-bucketed slot
    stream per (relation, index bank, 128-dst window, 8-dst subgroup).
    dma_gather (bf16, transpose=True) produces feature-major tiles; VectorE
    tensor_reduce over the innermost (slot) axis yields segment sums.  Pad
    slots point at an all-zero table row.  Tables over 32768 rows split into
    two int16 index banks; each bank gets its own per-window degree sort so
    subgroup depth padding stays low.
  - Per (relation, bank, window): a one-hot "unpermute * inv_deg" matrix is
    built by a fused tensor_scalar(is_equal, mult); two matmuls apply W_r and
    the window permutation back to natural dst order, accumulating into a
    feature-major fp32 SBUF accumulator (bank contributions sum there); the
    self-loop h @ L is one more matmul; bias+relu is a fused ScalarE
    activation per window.
"""
import sys
import types
import numpy as np
import ml_dtypes
from contextlib import ExitStack

import concourse.bass as bass
import concourse.bacc as bacc
import concourse.tile as tile
from concourse import mybir, library_config

BF16 = ml_dtypes.bfloat16
P = 128
SUBG = 16          # dsts per reduce subgroup
NSUB = P // SUBG   # subgroups per window
GCAP = 5120        # target max slots per dma_gather

CFG = dict(
    N={"drug": 20000, "gene": 50000, "disease": 10000},
    MOD={"drug": 1024, "gene": 768, "disease": 512},
    D_IN=128, D_H=128, D_OUT=64,
    RELS=[("drug", "disease", "dd"), ("drug", "drug", "ddr"),
          ("drug", "gene", "dg"), ("gene", "disease", "gd"),
          ("gene", "gene", "gg")],
    NCORE=8,
)

NTYPES = ("drug", "gene", "disease")
SRC_NTYPES = ("drug", "gene")
# processing order inside a layer: drug-dst first (unlocks drug AllGather),
# then gene-dst, then disease-dst (no AllGather needed).
REL_ORDER = ["ddr", "dg", "gg", "dd", "gd"]


# ---------------------------------------------------------------------------
# host-side preprocessing
# ---------------------------------------------------------------------------

def _pack_idx(stream):
    """int array (len % 128 == 0) -> dma_gather idx layout [128, len/16] int16:
    idx i at (i%16, i//16), replicated across the 8 groups of 16 partitions."""
    n = stream.size
    v = stream.astype(np.int16).reshape(n // 16, 16).T
    return np.tile(v, (8, 1))


def preprocess(cfg, inputs):
    ncore = cfg["NCORE"]
    shard = {nt: cfg["N"][nt] // ncore for nt in NTYPES}
    nw = {nt: -(-shard[nt] // P) for nt in NTYPES}

    S = dict(cfg=cfg, nw=nw, shard=shard, rels=[])
    percore = [dict() for _ in range(ncore)]

    BANK = 32768
    for r, (snt, dnt, tag) in enumerate(cfg["RELS"]):
        src = np.asarray(inputs["e_" + tag + "_s"]).astype(np.int64)
        dst = np.asarray(inputs["e_" + tag + "_d"]).astype(np.int64)
        trows = cfg["N"][snt] + 2
        # table row 0 is all-zero (pad target); banks split the int16 idx range
        if trows <= BANK:
            banks = [(0, trows)]
        else:
            banks = [(0, BANK), (BANK - 1, trows)]  # overlap row BANK-1 unused;
            # bank1 pad slots use relative row 0 -> absolute BANK-1 (a real
            # node!), so bank1 pads instead point at relative (trows-1-b0)
            # which is the trailing all-zero row.
        nbank = len(banks)
        NW = nw[dnt]
        dsh = shard[dnt]
        npad = NW * P

        core_of = dst // dsh
        deg_all = np.bincount(dst, minlength=cfg["N"][dnt]).astype(np.int64)
        row_all = src + 1
        bank_of = np.zeros(src.size, np.int64)
        for b, (b0, b1) in enumerate(banks):
            bank_of[(row_all >= b0 + (1 if b else 0)) & (row_all < b1)] = b
        pad_rel = [(trows - 1) - b0 for b0, b1 in banks]  # trailing zero row
        pad_rel[0] = 0                                    # leading zero row

        # per-bank, per-core window-local orderings by per-bank degree
        orders = np.zeros((nbank, ncore, npad), np.int64)
        perm_cols = np.zeros((nbank, ncore, NW, P), np.int32)
        invdeg_cols = np.zeros((nbank, ncore, NW, P), np.float32)
        dcnt = np.zeros((nbank, ncore, NW, P), np.int64)
        dn = np.arange(npad)
        for c in range(ncore):
            m = core_of == c
            ld_all = dst[m] - c * dsh
            deg_pad = np.zeros(npad, np.int64)
            deg_pad[:dsh] = deg_all[c * dsh:(c + 1) * dsh]
            ivfull = (1.0 / np.maximum(deg_pad, 1.0)).astype(np.float32)
            ivfull[dsh:] = 0.0
            for b in range(nbank):
                cb = np.bincount(ld_all[bank_of[m] == b], minlength=npad)
                order = np.lexsort((dn, -cb, dn // P))
                orders[b, c] = order
                perm_cols[b, c] = (order % P).reshape(NW, P)
                iv = ivfull[order].copy()
                iv[cb[order] == 0] = 0.0   # no bank-b edges -> contribute 0
                invdeg_cols[b, c] = iv.reshape(NW, P)
                dcnt[b, c] = cb[order].reshape(NW, P)

        # subgroup depths, common across cores; window block sizes % 128
        dq = np.zeros((NW, NSUB, nbank), np.int64)
        for q in range(NSUB):
            dq[:, q, :] = dcnt[:, :, :, q * SUBG:(q + 1) * SUBG].max(
                axis=(1, 3)).T

        # block layout: bank-major, then window, subgroup; greedy gathers.
        # Each gather is padded to a multiple of 128 slots at close time
        # (num_idxs % 128 requirement); pad slots belong to no block so
        # they are gathered but never reduced.
        blocks = []
        OFF = np.full((NW, NSUB, nbank), -1, np.int64)
        off = 0
        gathers = []

        def close_gather(b, gstart, off):
            off += (-(off - gstart)) % P
            gathers.append((b, gstart, off - gstart))
            return off

        for b in range(nbank):
            gstart, gslots = off, 0
            for w in range(NW):
                wslots = int(dq[w, :, b].sum()) * SUBG
                if wslots == 0:
                    continue
                if gslots + wslots > GCAP and gslots > 0:
                    off = close_gather(b, gstart, off)
                    gstart, gslots = off, 0
                for q in range(NSUB):
                    if dq[w, q, b] > 0:
                        blocks.append((w, b, q, int(dq[w, q, b]), off))
                        OFF[w, q, b] = off
                        off += int(dq[w, q, b]) * SUBG
                gslots += wslots
            if gslots > 0:
                off = close_gather(b, gstart, off)
        nslots = max(off, P)
        maxg = max((g[2] for g in gathers), default=P)

        for c in range(ncore):
            stream = np.zeros(nslots, np.int64)
            for (w, b, q, d, o) in blocks:
                stream[o:o + d * SUBG] = pad_rel[b]
            m = core_of == c
            sm_row = row_all[m]
            sm_bank = bank_of[m]
            ld = dst[m] - c * dsh
            for b, (b0, b1) in enumerate(banks):
                inb = sm_bank == b
                if not inb.any():
                    continue
                rel_row = sm_row[inb] - b0
                dp_of = np.zeros(npad, np.int64)
                dp_of[orders[b, c]] = np.arange(npad)
                e_dp = dp_of[ld[inb]]
                e_w, e_dpw = e_dp // P, e_dp % P
                e_q, e_i = e_dpw // SUBG, e_dpw % SUBG
                so = np.argsort(e_dp, kind="stable")
                ks = e_dp[so]
                starts = np.r_[0, np.flatnonzero(np.diff(ks)) + 1]
                sizes = np.diff(np.r_[starts, ks.size])
                cum = np.arange(ks.size) - np.repeat(starts, sizes)
                e_j = np.empty(ks.size, np.int64)
                e_j[so] = cum
                d_arr = dq[e_w, e_q, b]
                pos = OFF[e_w, e_q, b] + e_i * d_arr + e_j
                assert (pos >= 0).all() and (e_j < d_arr).all()
                stream[pos] = rel_row
            percore[c][f"idx_{tag}"] = _pack_idx(stream)
            # perm/invdeg stored [P, nbank*NW], bank-major columns
            percore[c][f"perm_{tag}"] = np.ascontiguousarray(
                perm_cols[:, c].astype(np.float32).reshape(
                    nbank * NW, P).T)
            percore[c][f"invdeg_{tag}"] = np.ascontiguousarray(
                invdeg_cols[:, c].reshape(nbank * NW, P).T)

        S["rels"].append(dict(r=r, snt=snt, dnt=dnt, tag=tag, NW=NW,
                              banks=banks, nbank=nbank, blocks=blocks,
                              gathers=gathers, nslots=nslots, maxg=maxg))

    for nt in NTYPES:
        x = np.asarray(inputs["x_" + nt])
        for c in range(ncore):
            sh = shard[nt]
            percore[c][f"xT_{nt}"] = np.ascontiguousarray(
                x[c * sh:(c + 1) * sh].T).astype(BF16)

    com = dict()
    for nt in NTYPES:
        com[f"We_{nt}"] = np.asarray(inputs["We_" + nt]).astype(BF16)
        com[f"be_{nt}"] = np.asarray(inputs["be_" + nt]).astype(
            np.float32).reshape(-1, 1)
    for l in range(3):
        com[f"W{l}"] = np.asarray(inputs[f"W{l}"]).astype(BF16)
        com[f"L{l}"] = np.asarray(inputs[f"L{l}"]).astype(BF16)
        com[f"b{l}"] = np.asarray(inputs[f"b{l}"]).astype(np.float32).reshape(-1, 1)
    com["iota"] = np.tile(np.arange(P, dtype=np.float32), (P, 1))
    for c in range(ncore):
        percore[c].update(com)
    return S, percore


# ---------------------------------------------------------------------------
# device program
# ---------------------------------------------------------------------------

def build(S):
    cfg = S["cfg"]
    ncore = cfg["NCORE"]
    nw, shard = S["nw"], S["shard"]
    DH, DOUT = cfg["D_H"], cfg["D_OUT"]
    NREL = len(cfg["RELS"])
    nsh_tot = sum(shard.values())
    maxg_all = max(R["maxg"] for R in S["rels"])
    maxw_cols = max(nw[nt] for nt in NTYPES) * P

    nc = bacc.Bacc("TRN2", target_bir_lowering=False, debug=False,
                   num_devices=ncore, dynamic_dma_scratch_size=24576)

    par = {}
    for nt in NTYPES:
        par[f"xT_{nt}"] = nc.declare_dram_parameter(
            f"xT_{nt}", [cfg["MOD"][nt], shard[nt]], mybir.dt.bfloat16, False)
        par[f"We_{nt}"] = nc.declare_dram_parameter(
            f"We_{nt}", [cfg["MOD"][nt], cfg["D_IN"]], mybir.dt.bfloat16, False)
        par[f"be_{nt}"] = nc.declare_dram_parameter(
            f"be_{nt}", [cfg["D_IN"], 1], mybir.dt.float32, False)
    for l in range(3):
        od = DOUT if l == 2 else DH
        par[f"W{l}"] = nc.declare_dram_parameter(
            f"W{l}", [NREL, DH, od], mybir.dt.bfloat16, False)
        par[f"L{l}"] = nc.declare_dram_parameter(
            f"L{l}", [DH, od], mybir.dt.bfloat16, False)
        par[f"b{l}"] = nc.declare_dram_parameter(
            f"b{l}", [od, 1], mybir.dt.float32, False)
    par["iota"] = nc.declare_dram_parameter("iota", [P, P], mybir.dt.float32, False)
    for R in S["rels"]:
        tg = R["tag"]
        par[f"idx_{tg}"] = nc.declare_dram_parameter(
            f"idx_{tg}", [P, R["nslots"] // 16], mybir.dt.int16, False)
        par[f"perm_{tg}"] = nc.declare_dram_parameter(
            f"perm_{tg}", [P, R["nbank"] * R["NW"]], mybir.dt.float32, False)
        par[f"invdeg_{tg}"] = nc.declare_dram_parameter(
            f"invdeg_{tg}", [P, R["nbank"] * R["NW"]], mybir.dt.float32, False)
    out_par = nc.declare_dram_parameter("out", [nsh_tot, DOUT],
                                        mybir.dt.float32, True)

    agin, tabs = {}, {}
    for l in range(3):
        for nt in SRC_NTYPES:
            agin[(l, nt)] = nc.dram_tensor(
                f"agin{l}_{nt}", [shard[nt], DH], mybir.dt.bfloat16)
            tabs[(l, nt)] = nc.dram_tensor(
                f"tab{l}_{nt}", [cfg["N"][nt] + 2, DH], mybir.dt.bfloat16,
                addr_space="Shared")

    rel_by_tag = {R["tag"]: R for R in S["rels"]}

    with ExitStack() as ctx:
        tc = ctx.enter_context(tile.TileContext(nc))
        nc.gpsimd.load_library(library_config.mlp)

        const = ctx.enter_context(tc.tile_pool(name="const", bufs=1))
        persist = ctx.enter_context(tc.tile_pool(name="persist", bufs=1))
        gpool = ctx.enter_context(tc.tile_pool(name="gpool", bufs=4))
        ipool = ctx.enter_context(tc.tile_pool(name="ipool", bufs=4))
        xpool = ctx.enter_context(tc.tile_pool(name="xpool", bufs=2))
        wpool = ctx.enter_context(tc.tile_pool(name="wpool", bufs=4))
        prpool = ctx.enter_context(tc.tile_pool(name="prpool", bufs=2))
        pst = ctx.enter_context(tc.tile_pool(name="pst", bufs=2, space="PSUM"))
        ps1 = ctx.enter_context(tc.tile_pool(name="ps1", bufs=2, space="PSUM"))
        ps2 = ctx.enter_context(tc.tile_pool(name="ps2", bufs=2, space="PSUM"))
        psE = ctx.enter_context(tc.tile_pool(name="psE", bufs=2, space="PSUM"))

        sb_iota = const.tile([P, P], mybir.dt.float32)
        nc.sync.dma_start(sb_iota[:], par["iota"][:])
        identity = const.tile([P, P], mybir.dt.float32)
        from concourse.masks import make_identity
        make_identity(nc, identity[:])
        identity16 = const.tile([P, P], mybir.dt.bfloat16)
        nc.vector.tensor_copy(identity16[:], identity[:])

        sb_W, sb_L, sb_b = {}, {}, {}
        for l in range(3):
            od = DOUT if l == 2 else DH
            t = const.tile([DH, NREL, od], mybir.dt.bfloat16, tag=f"W{l}")
            nc.sync.dma_start(t[:], par[f"W{l}"][:].rearrange("r k o -> k r o"))
            sb_W[l] = t
            sb_L[l] = const.tile([DH, od], mybir.dt.bfloat16, tag=f"L{l}", name=f"L{l}")
            nc.sync.dma_start(sb_L[l][:], par[f"L{l}"][:])
            sb_b[l] = const.tile([od, 1], mybir.dt.float32, tag=f"b{l}", name=f"b{l}")
            nc.sync.dma_start(sb_b[l][:], par[f"b{l}"][:])
        sb_meta = {}
        for R in S["rels"]:
            tg = R["tag"]
            pm = const.tile([P, R["nbank"] * R["NW"]], mybir.dt.float32,
                            tag=f"pm_{tg}")
            nc.sync.dma_start(pm[:], par[f"perm_{tg}"][:])
            iv = const.tile([P, R["nbank"] * R["NW"]], mybir.dt.float32,
                            tag=f"iv_{tg}")
            nc.sync.dma_start(iv[:], par[f"invdeg_{tg}"][:])
            sb_meta[tg] = (pm, iv)

        zrow = const.tile([1, DH], mybir.dt.bfloat16)
        nc.vector.memset(zrow[:], 0.0)
        for l in range(3):
            for nt in SRC_NTYPES:
                n = cfg["N"][nt]
                nc.sync.dma_start(tabs[(l, nt)][0:1, :], zrow[:])
                nc.sync.dma_start(tabs[(l, nt)][n + 1:n + 2, :], zrow[:])

        hT = [persist.tile([DH, nsh_tot], mybir.dt.bfloat16, tag=f"hT{i}",
                           name=f"hT{i}")
              for i in range(2)]
        nt_off, o = {}, 0
        for nt in NTYPES:
            nt_off[nt] = o
            o += shard[nt]
        agg = persist.tile([DH, nsh_tot], mybir.dt.float32, tag="agg")

        def emit_ag(l, nt):
            """Transpose this core's h shard of ntype nt and AllGather it
            into the layer-l gather table."""
            sh = shard[nt]
            for w0 in range(0, sh, P):
                cols = min(P, sh - w0)
                src = hT[l % 2][:, nt_off[nt] + w0:nt_off[nt] + w0 + cols]
                pt = pst.tile([P, P], mybir.dt.bfloat16, tag="tp", name="pt16")
                nc.tensor.transpose(pt[:cols, :DH], src, identity16[:])
                stg = wpool.tile([P, DH], mybir.dt.bfloat16, tag="agstg")
                nc.scalar.activation(stg[:cols, :], pt[:cols, :DH],
                                     mybir.ActivationFunctionType.Identity)
                nc.sync.dma_start(agin[(l, nt)][w0:w0 + cols, :],
                                  stg[:cols, :])
            nc.gpsimd.collective_compute(
                "AllGather", mybir.AluOpType.bypass,
                replica_groups=[list(range(ncore))],
                ins=[agin[(l, nt)][:]],
                outs=[tabs[(l, nt)][1:cfg["N"][nt] + 1]],
            )

        def emit_embedding():
            for nt in ("drug", "gene", "disease"):
                mod, sh = cfg["MOD"][nt], shard[nt]
                kt = mod // P
                sb_we = xpool.tile([P, 8, cfg["D_IN"]], mybir.dt.bfloat16, tag="we")
                nc.sync.dma_start(
                    sb_we[:, :kt, :],
                    par[f"We_{nt}"][:].rearrange("(k p) f -> p k f", p=P))
                sb_be = wpool.tile([cfg["D_IN"], 1], mybir.dt.float32, tag="be")
                nc.sync.dma_start(sb_be[:], par[f"be_{nt}"][:])
                for n0 in range(0, sh, 512):
                    n1 = min(n0 + 512, sh)
                    cols = n1 - n0
                    xt = xpool.tile([P, 8, 512], mybir.dt.bfloat16, tag="xt")
                    nc.sync.dma_start(
                        xt[:, :kt, :cols],
                        par[f"xT_{nt}"][:].rearrange(
                            "(k p) n -> p k n", p=P)[:, :, n0:n1])
                    pe = psE.tile([P, 512], mybir.dt.float32, tag="emb")
                    for k in range(kt):
                        nc.tensor.matmul(pe[:, :cols], sb_we[:, k, :],
                                         xt[:, k, :cols],
                                         start=(k == 0), stop=(k == kt - 1))
                    nc.scalar.activation(
                        hT[0][:, nt_off[nt] + n0:nt_off[nt] + n1], pe[:, :cols],
                        mybir.ActivationFunctionType.Identity, bias=sb_be[:])
                if nt in SRC_NTYPES:
                    emit_ag(0, nt)

        def emit_relation(l, R):
            """Per bank: gathers + segment sums into praw, then per-window
            W_r + unpermute*invdeg matmuls accumulated into agg columns of
            R's dst ntype (bank contributions sum in agg)."""
            od = DOUT if l == 2 else DH
            tg, snt, dnt, r, NW = R["tag"], R["snt"], R["dnt"], R["r"], R["NW"]
            tab = tabs[(l, snt)]
            pm, iv = sb_meta[tg]
            blk_by_g = {gi: [] for gi in range(len(R["gathers"]))}
            for blk in R["blocks"]:
                for gi, (gb, goff, gslots) in enumerate(R["gathers"]):
                    if gb == blk[1] and goff <= blk[4] < goff + gslots:
                        blk_by_g[gi].append(blk)
                        break
            for b, (b0, b1) in enumerate(R["banks"]):
                praw = prpool.tile([P, maxw_cols], mybir.dt.bfloat16,
                                   tag="praw")
                written = np.zeros((NW, NSUB), bool)
                for gi, (gb, goff, gslots) in enumerate(R["gathers"]):
                    if gb != b:
                        continue
                    sbi = ipool.tile([P, maxg_all // 16], mybir.dt.int16,
                                     tag="idx")
                    nc.sync.dma_start(
                        sbi[:, :gslots // 16],
                        par[f"idx_{tg}"][:, goff // 16:(goff + gslots) // 16])
                    gt = gpool.tile([P, 1, maxg_all], mybir.dt.bfloat16,
                                    tag="gat")
                    nc.gpsimd.dma_gather(
                        out_ap=gt[:, :, :gslots], in_ap=tab[b0:b1],
                        idxs_ap=sbi[:, :gslots // 16],
                        num_idxs=gslots, num_idxs_reg=gslots,
                        elem_size=DH, transpose=True,
                        single_packet=(gslots <= 992))
                    for (w, bb, q, d, off) in blk_by_g[gi]:
                        loc = off - goff
                        view = gt[:, 0, loc:loc + d * SUBG].rearrange(
                            "p (n d) -> p n d", d=d)
                        cols = slice(w * P + q * SUBG, w * P + (q + 1) * SUBG)
                        with nc.allow_low_precision(
                                reason="DVE reduces in fp32; bf16 rounding "
                                       "applies once at output"):
                            nc.vector.tensor_reduce(
                                praw[:, cols], view, axis=mybir.AxisListType.X,
                                op=mybir.AluOpType.add)
                        written[w, q] = True
                for w in range(NW):
                    for q in range(NSUB):
                        if not written[w, q]:
                            nc.vector.memset(
                                praw[:, w * P + q * SUBG:
                                     w * P + (q + 1) * SUBG], 0.0)
                for w in range(NW):
                    wb = b * NW + w
                    Sp = wpool.tile([P, P], mybir.dt.bfloat16, tag="Sperm")
                    nc.vector.tensor_scalar(
                        Sp[:], sb_iota[:], pm[:, wb:wb + 1], iv[:, wb:wb + 1],
                        op0=mybir.AluOpType.is_equal, op1=mybir.AluOpType.mult)
                    p1 = ps1.tile([P, DH], mybir.dt.float32, tag="out1")
                    nc.tensor.matmul(p1[:, :od], praw[:, w * P:(w + 1) * P],
                                     sb_W[l][:, r, :],
                                     start=True, stop=True)
                    o1 = wpool.tile([P, DH], mybir.dt.bfloat16, tag="o1")
                    nc.scalar.activation(
                        o1[:, :od], p1[:, :od],
                        mybir.ActivationFunctionType.Identity)
                    p2 = ps2.tile([P, P], mybir.dt.float32, tag="out2")
                    nc.tensor.matmul(p2[:od, :], o1[:, :od], Sp[:],
                                     start=True, stop=True)
                    cs = nt_off[dnt] + w * P
                    ce = min(cs + P, nt_off[dnt] + shard[dnt])
                    nc.vector.tensor_add(agg[:od, cs:ce], agg[:od, cs:ce],
                                         p2[:od, :ce - cs])

        def emit_finish_ntype(l, nt):
            """Self-loop + bias (+relu) for ntype nt; write hT (or output)."""
            od = DOUT if l == 2 else DH
            sh = shard[nt]
            for w0 in range(0, sh, P):
                cols = min(P, sh - w0)
                cs = nt_off[nt] + w0
                p2 = ps2.tile([P, P], mybir.dt.float32, tag="out2")
                nc.tensor.matmul(p2[:od, :cols], sb_L[l][:],
                                 hT[l % 2][:, cs:cs + cols],
                                 start=True, stop=True)
                nc.vector.tensor_add(agg[:od, cs:cs + cols],
                                     agg[:od, cs:cs + cols],
                                     p2[:od, :cols])
                if l < 2:
                    nc.scalar.activation(
                        hT[(l + 1) % 2][:od, cs:cs + cols],
                        agg[:od, cs:cs + cols],
                        mybir.ActivationFunctionType.Relu, bias=sb_b[l][:])
                else:
                    fin = wpool.tile([P, P], mybir.dt.float32, tag="fin")
                    nc.scalar.activation(
                        fin[:od, :cols], agg[:od, cs:cs + cols],
                        mybir.ActivationFunctionType.Identity,
                        bias=sb_b[l][:])
                    pt = pst.tile([P, P], mybir.dt.float32, tag="tp")
                    nc.tensor.transpose(pt[:cols, :od], fin[:od, :cols],
                                        identity[:od, :od])
                    stg = wpool.tile([P, DOUT], mybir.dt.float32, tag="ostg")
                    nc.vector.tensor_copy(stg[:cols, :], pt[:cols, :od])
                    nc.sync.dma_start(out_par[cs:cs + cols, :],
                                      stg[:cols, :])
            if l < 2 and nt in SRC_NTYPES:
                emit_ag(l + 1, nt)

        def emit_layer(l):
            od = DOUT if l == 2 else DH
            # first relation targeting each dst ntype clears its agg columns
            cleared = set()
            # ntype completed when all relations targeting it are done
            remaining = {nt: sum(1 for R in S["rels"] if R["dnt"] == nt)
                         for nt in NTYPES}
            for tg in REL_ORDER:
                R = rel_by_tag[tg]
                dnt = R["dnt"]
                if dnt not in cleared:
                    cs, sh = nt_off[dnt], shard[dnt]
                    nc.vector.memset(agg[:od, cs:cs + sh], 0.0)
                    cleared.add(dnt)
                emit_relation(l, R)
                remaining[dnt] -= 1
                if remaining[dnt] == 0:
                    emit_finish_ntype(l, dnt)

        emit_embedding()
        emit_layer(0)
        emit_layer(1)
        emit_layer(2)

    nc.compile()
    return nc


# ---------------------------------------------------------------------------
# entry point
# ---------------------------------------------------------------------------

def _install_ntff_hook():
    if "antenv.axon_hooks" in sys.modules:
        return
    mod = types.ModuleType("antenv.axon_hooks")
    mod._hook = None
    mod.set_axon_ntff_profile_hook = lambda h: setattr(mod, "_hook", h)
    mod.get_axon_ntff_profile_hook = lambda: mod._hook
    sys.modules["antenv.axon_hooks"] = mod
    try:
        import antenv
        antenv.axon_hooks = mod
        from trn_agent_boot.trn_boot import _ntff_profile_via_ctypes
        hook = _ntff_profile_via_ctypes("/opt/axon/libaxon_pjrt.so")
        if hook is not None:
            mod.set_axon_ntff_profile_hook(hook)
    except Exception:
        pass


def run(inputs, cfg=CFG, trace=False, tmpdir=None):
    S, percore = preprocess(cfg, inputs)
    nc = build(S)
    _install_ntff_hook()
    from concourse import bass_utils
    bass_utils.upload_artifacts = lambda d: d
    res = bass_utils.run_bass_kernel_spmd(
        nc, percore, list(range(cfg["NCORE"])), trace=trace, tmpdir=tmpdir,
        trace_cores=[0] if trace else None)
    ncore = cfg["NCORE"]
    shard = {nt: cfg["N"][nt] // ncore for nt in NTYPES}
    outs = []
    o = 0
    for nt in NTYPES:
        parts = [res.results[c]["out"][o:o + shard[nt]] for c in range(ncore)]
        outs.append(np.concatenate(parts, 0))
        o += shard[nt]
    full = np.concatenate(outs, 0).astype(np.float32)
    run.last_exec_time_ns = res.exec_time_ns
    return full


def kernel(**inputs):
    return run(inputs)
